# revision 1
# baseline (speedup 1.0000x reference)
"""Trainium2 Bass kernel for nn_Block_55448027791422 (dense transformer block).

Strategy: pure data-parallel over batch B=16 across 8 NeuronCores (2 batches
per core), zero collectives.  Activations live on-chip channel-major (C on
partitions, tokens on free dim).  v2 changes vs the bf16 baseline:
  - qkv / v / proj matmuls run fp8(e4m3) DoubleRow (2x PE throughput);
    n1 / o / n2 activations are stored as fp8 channel-pair tiles.
  - rel_bias is ADDED into the score psum via an identity-stationary matmul
    (raw bf16 rel_bias streamed from HBM), so softmax needs no separate
    exp(bias) multiply on the Vector engine.
  - exp writes probs directly as fp8 (shifted by exp(-6.5) for range), and
    attn@V runs fp8 DoubleRow over key-tile pairs.
  - LN2 is emitted text-chunk-first so the text MLP starts while the image
    LN2 elementwise tail overlaps it; image-MLP token tails of both batches
    are merged into single matmuls via 3D/4D access patterns.
"""

import os
import sys
import contextlib
import ctypes
import types

import numpy as np

for _p in ("/opt/trn_rl_repo",):
    if _p not in sys.path:
        sys.path.insert(0, _p)

import ml_dtypes

bfloat16 = ml_dtypes.bfloat16

# ---------------------------------------------------------------- constants
B, N, C, H, HD, HID, TXT = 16, 616, 768, 12, 64, 3072, 40
NCORES = 8
BL = B // NCORES            # 2 batches per core
NT = BL * N                 # 1232 tokens per core
KT = C // 128               # 6 channel tiles
KTP = KT // 2               # 3 channel-pair tiles (fp8 DoubleRow)
MT_QK = (2 * C) // 128      # 12 output tiles for q,k
KT_HID = HID // 128         # 24 hidden tiles
EPS = 1e-5
KEY_TILES = [(0, 128), (128, 128), (256, 128), (384, 128), (512, 104)]
NKT = len(KEY_TILES)
SCALE = HD ** -0.5
SW_Q, SW_K, SW_V, SW_P = 128.0, 16.0, 16.0, 16.0   # fp8 weight pre-scales
EXP_SHIFT = 6.5             # probs = exp(score - EXP_SHIFT); cancels in ratio
VHW = 80                    # per-head column pitch in the V tiles (65 used)


def _chunks(total, step=512, base=0):
    out, o = [], 0
    while o < total:
        s = min(step, total - o)
        out.append((base + o, s))
        o += s
    return out


NT_CH = _chunks(NT)                      # [(0,512),(512,512),(1024,208)]
Q_CH = [(0, 512), (512, 104)]            # per-batch query chunks


# const-vector column indexes in the [128, NCONST] consts tensor
def _cc():
    idx = {}
    c = 0
    for name, n in [("bqk", MT_QK), ("g1", KT), ("g1bp", KT),
                    ("g2", KT), ("g2bt2", KT), ("g2bi2", KT),
                    ("bt1", KT_HID), ("bi1", KT_HID)]:
        idx[name] = c
        c += n
    return idx, c


CCOL, NCONST = _cc()

_ENV_TRACE = "BASS_KERNEL_TRACE"
LAST_EXEC_TIME_NS = None
LAST_TRACE_PATH = None


# ------------------------------------------------------- axon profile hook
def _install_ntff_hook():
    """run_bass_kernel_spmd(trace=True) under axon needs antenv.axon_hooks."""
    if "antenv.axon_hooks" in sys.modules:
        return
    so_path = "/opt/axon/libaxon_pjrt.so"
    state = {"h": None}

    def _build():
        try:
            lib = ctypes.CDLL(so_path)
        except OSError:
            return None
        if not hasattr(lib, "axon_start_nrt_profile"):
            return None
        lib.axon_start_nrt_profile.argtypes = [ctypes.POINTER(ctypes.c_int64),
                                               ctypes.c_size_t]
        lib.axon_start_nrt_profile.restype = ctypes.c_int64
        lib.axon_stop_nrt_profile.argtypes = [ctypes.c_char_p]
        lib.axon_stop_nrt_profile.restype = ctypes.c_int64

        @contextlib.contextmanager
        def _hook(output_dir, device_ids):
            import jax
            jax.devices()
            if device_ids:
                ids = (ctypes.c_int64 * len(device_ids))(*device_ids)
                rc = lib.axon_start_nrt_profile(ids, len(device_ids))
            else:
                rc = lib.axon_start_nrt_profile(None, 0)
            if rc != 0:
                raise RuntimeError(f"axon_start_nrt_profile rc={rc}")
            try:
                yield
            finally:
                n = lib.axon_stop_nrt_profile(str(output_dir).encode())
                if n < 0:
                    raise RuntimeError(f"axon_stop_nrt_profile rc={n}")

        return _hook

    def get_axon_ntff_profile_hook():
        if state["h"] is None:
            state["h"] = _build()
        return state["h"]

    mod = types.ModuleType("antenv.axon_hooks")
    mod.get_axon_ntff_profile_hook = get_axon_ntff_profile_hook
    mod.set_axon_ntff_profile_hook = lambda h: state.update(h=h)
    sys.modules["antenv.axon_hooks"] = mod


# ------------------------------------------------------------ graph builder
_NC_CACHE = {}


def _build_nc():
    import concourse.bass as bass  # noqa: F401
    import concourse.mybir as mybir
    import concourse.tile as tile
    from concourse import bacc

    F32 = mybir.dt.float32
    BF16 = mybir.dt.bfloat16
    F8 = mybir.dt.float8e4
    AF = mybir.ActivationFunctionType
    DR = mybir.MatmulPerfMode.DoubleRow

    nc = bacc.Bacc(None, target_bir_lowering=False)
    d = nc.declare_dram_parameter
    x_d = d("x", [BL, C, N], F32, isOutput=False)
    eb_d = d("relbias", [H, NKT, 128, N], BF16, isOutput=False)
    wqk_d = d("wqk", [KTP, 128, 2, 2 * C], F8, isOutput=False)
    wv_d = d("wv", [KTP, 128, 2, C], F8, isOutput=False)
    wproj_d = d("wproj", [KTP, 128, 2, C], F8, isOutput=False)
    id_d = d("id128", [128, 128], BF16, isOutput=False)
    wt1_d = d("wt1", [C, HID], F8, isOutput=False)
    wi1_d = d("wi1", [3, 128, 2, HID], F8, isOutput=False)
    wt2_d = d("wt2", [HID, C], F8, isOutput=False)
    wi2_d = d("wi2", [12, 128, 2, C], F8, isOutput=False)
    consts_d = d("consts", [128, NCONST], F32, isOutput=False)
    out_d = d("out", [BL, C, N], F32, isOutput=True)

    def p3(t):
        """[128, NT] view -> [128, BL, N] batch-split view."""
        return t.rearrange("p (b n) -> p b n", b=BL)

    with tile.TileContext(nc) as tc:
        with contextlib.ExitStack() as octx:
            per = octx.enter_context(tc.tile_pool(name="perm", bufs=1))
            # persistent tiles
            consts = per.tile([128, NCONST], F32, tag="consts")

            def cvec(name, i):
                return consts[:, CCOL[name] + i:CCOL[name] + i + 1]

            nc.sync.dma_start(out=consts[:], in_=consts_d[:])
            ones128 = per.tile([128, 128], BF16, tag="ones128")
            nc.vector.memset(ones128[:], 1.0)
            id128 = per.tile([128, 128], BF16, tag="id128")
            nc.sync.dma_start(out=id128[:], in_=id_d[:])
            eps_ap = per.tile([128, 1], F32, tag="epsap")
            nc.vector.memset(eps_ap[:], EPS)
            shift_ap = per.tile([128, 1], F32, tag="shiftap")
            nc.vector.memset(shift_ap[:], -float(EXP_SHIFT))

            x_sb = [per.tile([128, NT], F32, tag="x", bufs=KT, name=f"x{i}")
                    for i in range(KT)]
            for kt in range(KT):
                for b in range(BL):
                    nc.sync.dma_start(
                        out=x_sb[kt][:, b * N:(b + 1) * N],
                        in_=x_d[b, kt * 128:(kt + 1) * 128, :])

            # fp8 channel-pair activation tiles (DoubleRow operands):
            # pair t holds channel tiles (2t, 2t+1) in slots (0, 1)
            n1p = [per.tile([128, 2, NT], F8, tag="n1p", bufs=KTP,
                            name=f"n1p{i}") for i in range(KTP)]
            op = [per.tile([128, 2, NT], F8, tag="op", bufs=KTP,
                           name=f"op{i}") for i in range(KTP)]
            # V tiles: per (batch, key tile); per-head pitch VHW,
            # col h*VHW+64 holds the softmax-denominator ones column.
            vp5 = [[per.tile([128, H * VHW], F8, tag="vp", bufs=BL * NKT,
                             name=f"vp{b}_{i}") for i in range(NKT)]
                   for b in range(BL)]

            n2_pair = [per.tile([128, 2, NT], F8, tag="n2p", bufs=KTP,
                                name=f"n2p{i}") for i in range(KTP)]
            n2_sb = [n2_pair[i // 2][:, i % 2, :] for i in range(KT)]

            # LN scratch lives in the outer pool (used by both phases)
            def ln_scratch(tag, w, dt, bufs, name):
                return per.tile([128, 512], dt, tag=tag, bufs=bufs,
                                name=name)[:, :w]

            # ------------------------------------------------ LayerNorm
            def ln_stats(x_tiles, start, w, pair, psm_fn, pse_fn):
                """Stats for one token chunk -> (mn, r) [128, fw] tiles.
                pair=True: b-symmetric range [start,start+w) of BOTH batches
                (within-batch offset), free size 2w.  Else NT-offset."""
                fw = 2 * w if pair else w

                def src(kt):
                    if pair:
                        return p3(x_tiles[kt])[:, :, start:start + w]
                    return x_tiles[kt][:, start:start + w]

                ps_m = psm_fn(fw)
                ps_e = pse_fn(fw)
                for kt in range(KT):
                    xbt = ln_scratch("xbt", fw, BF16, 3, "xbt")
                    sqt = ln_scratch("sqt", fw, BF16, 3, "sqt")
                    xbt_v = (xbt.rearrange("p (b n) -> p b n", b=2)
                             if pair else xbt)
                    sqt_v = (sqt.rearrange("p (b n) -> p b n", b=2)
                             if pair else sqt)
                    nc.vector.tensor_copy(xbt_v, src(kt))
                    if kt % 2 == 0:
                        nc.scalar.activation(sqt_v, src(kt), AF.Square)
                    else:
                        nc.gpsimd.tensor_mul(sqt_v, src(kt), src(kt))
                    nc.tensor.matmul(ps_m, ones128[:], xbt,
                                     start=(kt == 0), stop=(kt == KT - 1))
                    nc.tensor.matmul(ps_e, ones128[:], sqt,
                                     start=(kt == 0), stop=(kt == KT - 1))
                m2 = ln_scratch("lntmp", fw, F32, 3, "m2")
                nc.scalar.activation(m2, ps_m, AF.Square,
                                     scale=float(C ** -0.5))
                dd = ln_scratch("lntmp", fw, F32, 3, "dd")
                nc.vector.tensor_sub(dd, ps_e, m2)
                s = ln_scratch("lntmp", fw, F32, 3, "s")
                nc.scalar.activation(s, dd, AF.Sqrt, bias=eps_ap[:, 0:1],
                                     scale=float(1.0 / C))
                r = ln_scratch("lnr", fw, F32, 8, "r")
                nc.vector.reciprocal_approx_fast(out=r, in_=s)
                mn = ln_scratch("lnr", fw, F32, 8, "mn")
                nc.scalar.mul(mn, ps_m, float(1.0 / C))
                return mn, r

            def ln_elem(x_tiles, out_writer, start, w, pair, mn, r):
                fw = 2 * w if pair else w

                def src(kt):
                    if pair:
                        return p3(x_tiles[kt])[:, :, start:start + w]
                    return x_tiles[kt][:, start:start + w]

                mn_v = (mn.rearrange("p (b n) -> p b n", b=2)
                        if pair else mn)
                r_v = r.rearrange("p (b n) -> p b n", b=2) if pair else r
                for kt in range(KT):
                    t = ln_scratch("lnt", fw, F32, 4, "t")
                    tv = (t.rearrange("p (b n) -> p b n", b=2)
                          if pair else t)
                    e1, e2 = ((nc.gpsimd, nc.vector) if kt % 2 == 0
                              else (nc.vector, nc.gpsimd))
                    e1.tensor_sub(tv, src(kt), mn_v)
                    dst = out_writer(kt, start, w, pair)
                    e2.tensor_mul(dst, tv, r_v)

            # ---------------- early pool (qkv / attention / proj / stats)
            with contextlib.ExitStack() as ectx:
                ep = ectx.enter_context(tc.tile_pool(name="early", bufs=1))
                psA = ectx.enter_context(
                    tc.tile_pool(name="psA", bufs=1, space="PSUM"))

                def psS(w=616):
                    return psA.tile([128, 616], F32, tag="S", bufs=2,
                                    name="psS")[:, :w]

                def psACC(w=616):
                    return psA.tile([128, 616], F32, tag="acc", bufs=2,
                                    name="psAcc")[:, :w]

                # early-dying tags first so the mlp pool can reuse space
                wqk_sb = [ep.tile([128, 2, 2 * C], F8, tag="wqk", bufs=KTP,
                                  name=f"wqk{i}") for i in range(KTP)]
                _d2 = ep.tile([128, 2, 624], F8, tag="prp", bufs=8, name="d2")
                _d3 = ep.tile([128, 624], F8, tag="prs", bufs=4, name="d3")
                _d4 = ep.tile([128, N], BF16, tag="eb", bufs=15, name="d4")
                _d5 = ep.tile([1, N], F32, tag="den", bufs=4, name="d5")
                _d6 = ep.tile([64, N], F32, tag="recb", bufs=2, name="d6")
                wv_sb = [ep.tile([128, 2, C], F8, tag="wv", bufs=KTP,
                                 name=f"wv{i}") for i in range(KTP)]
                wproj_sb = [ep.tile([128, 2, C], F8, tag="wproj", bufs=KTP,
                                    name=f"wpj{i}") for i in range(KTP)]
                for t in range(KTP):
                    nc.sync.dma_start(out=wqk_sb[t][:], in_=wqk_d[t])
                    nc.sync.dma_start(out=wv_sb[t][:], in_=wv_d[t])
                for t in range(KTP):
                    nc.sync.dma_start(out=wproj_sb[t][:], in_=wproj_d[t])

                qk_sb = [ep.tile([128, NT], BF16, tag="qk", bufs=MT_QK,
                                 name=f"qk{i}") for i in range(MT_QK)]

                # -------------------------------------------------- LN1
                def n1_writer(kt, start, w, pair):
                    dst = n1p[kt // 2][:, kt % 2, :]
                    if pair:
                        return p3(dst)[:, :, start:start + w]
                    return dst[:, start:start + w]

                for (o, w) in NT_CH:
                    mn, r = ln_stats(x_sb, o, w, False, psS, psACC)
                    ln_elem(x_sb, n1_writer, o, w, False, mn, r)

                # ------------------------------------------------- v matmul
                # vpA: key tiles (0,1); vpB: (2,3); vpC: 4.  fp8 DoubleRow
                # over channel pairs; psum = 16*v -> copy scale 1/16.
                for b in range(BL):
                    for ktl in range(NKT):
                        vt = vp5[b][ktl]
                        nc.vector.memset(vt[:], 0.0)
                        nc.vector.memset(
                            vt.rearrange("p (h e) -> p h e", e=VHW)
                            [:, :, 64:65], 1.0)
                for b in range(BL):
                    for ktl, (koff, ksz) in enumerate(KEY_TILES):
                        toff = b * N + koff
                        vt3 = vp5[b][ktl].rearrange("p (h e) -> p h e",
                                                    e=VHW)
                        vch = [(0, 512), (512, 256)]
                        pss = [(psS(512) if ci == 0 else psACC(256))[:ksz, :]
                               for ci in range(2)]
                        # t-outer: one stationary load serves both chunks
                        for t in range(KTP):
                            for ci, (o, w) in enumerate(vch):
                                nc.tensor.matmul(
                                    pss[ci], n1p[t][:, :, toff:toff + ksz],
                                    wv_sb[t][:, :, o:o + w],
                                    start=(t == 0), stop=(t == KTP - 1),
                                    perf_mode=DR)
                        for ci, (o, w) in enumerate(vch):
                            nheads = w // 64
                            h0 = o // 64
                            nc.scalar.activation(
                                vt3[:ksz, h0:h0 + nheads, 0:64],
                                pss[ci].rearrange("p (h e) -> p h e", e=64),
                                AF.Copy, scale=float(1.0 / SW_V))

                # ------------------------------------------------ q,k matmul
                def emit_qk(mt, mi):
                    msl = slice(mt * 128, (mt + 1) * 128)
                    sw = SW_Q if mt < KT else SW_K
                    pss = []
                    for ci, (o, w) in enumerate(NT_CH):
                        ps = (psS(w) if (mi + ci) % 2 == 0 else psACC(w))
                        pss.append((ps, o, w))
                    # t-outer: one stationary load serves all three chunks
                    for t in range(KTP):
                        for ps, o, w in pss:
                            nc.tensor.matmul(ps, wqk_sb[t][:, :, msl],
                                             n1p[t][:, :, o:o + w],
                                             start=(t == 0),
                                             stop=(t == KTP - 1),
                                             perf_mode=DR)
                    for ps, o, w in pss:
                        nc.scalar.activation(qk_sb[mt][:, o:o + w], ps,
                                             AF.Identity,
                                             bias=cvec("bqk", mt),
                                             scale=float(1.0 / sw))

                # only the first head-pair's q/k upfront; the rest are
                # emitted inside the attention loop one pair ahead, so the
                # qkv matmuls fill attention's exp-bound PE stalls
                emit_qk(0, 0)
                emit_qk(6, 1)

                # ------------------------------------------------ attention
                eb_pool = [None] * (H * NKT)

                def eb_tile(h, ktl):
                    i = h * NKT + ktl
                    if eb_pool[i] is None:
                        t = ep.tile([128, N], BF16, tag="eb", bufs=15,
                                    name=f"eb{i}")
                        nc.sync.dma_start(out=t[:], in_=eb_d[h, ktl])
                        eb_pool[i] = t
                    return eb_pool[i]

                def scores_probs_pair(hp, b, rewarm=False):
                    """Scores for heads (2hp, 2hp+1) of batch b.  The two
                    heads' q/k live in rows 0-63 / 64-127 of the same tiles,
                    so their qk matmuls land in disjoint PE row groups and
                    run concurrently when issued back-to-back.  psum =
                    rel_bias (identity matmul) + q.k; praw = exp(s-shift)
                    as fp8 pairs, one set per head."""
                    qt = qk_sb[hp]
                    kt_t = qk_sb[KT + hp]
                    prz = []
                    for _ in range(2):
                        prp = [ep.tile([128, 2, 624], F8, tag="prp", bufs=8,
                                       name="prp") for _ in range(2)]
                        prs = ep.tile([128, 624], F8, tag="prs", bufs=4,
                                      name="prs")
                        prz.append((prp, prs))
                    for ktl, (koff, ksz) in enumerate(KEY_TILES):
                        pz = [psS(), psS()]
                        ebz = [eb_tile(2 * hp, ktl),
                               eb_tile(2 * hp + 1, ktl)]
                        if ktl == 0:
                            # full-array warmup keeps the PE clock-gate
                            # open; periodic ~2.5us bursts force a fully-
                            # busy HAM window so a transient stall cannot
                            # latch the attention phase at half clock.
                            nburst = 12 if rewarm else 1
                            for _ in range(nburst):
                                nc.tensor.matmul(pz[0][:, 0:512], ones128[:],
                                                 qk_sb[0][:, 0:512],
                                                 start=True, stop=True)
                        ksl = slice(b * N + koff, b * N + koff + ksz)
                        for i in range(2):
                            for (qo, qw) in Q_CH:
                                nc.tensor.matmul(
                                    pz[i][:ksz, qo:qo + qw], id128[:, :ksz],
                                    ebz[i][:, qo:qo + qw],
                                    start=True, stop=False)
                        # adjacent qk matmuls: rows 0-63 then 64-127
                        for i in range(2):
                            po = i * 64
                            for (qo, qw) in Q_CH:
                                nc.tensor.matmul(
                                    pz[i][:ksz, qo:qo + qw],
                                    kt_t[po:po + 64, ksl],
                                    qt[po:po + 64,
                                       b * N + qo:b * N + qo + qw],
                                    start=False, stop=True)
                        for i in range(2):
                            prp, prs = prz[i]
                            dst = (prp[ktl // 2][:, ktl % 2, :N] if ktl < 4
                                   else prs[:, :N])
                            nc.scalar.activation(dst[:ksz, :],
                                                 pz[i][:ksz, :], AF.Exp,
                                                 bias=shift_ap[:ksz, 0:1])
                    return prz

                def attnv(h, b, prp, prs):
                    vcol = slice(h * VHW, h * VHW + 65)
                    pv = psACC()[:65, :]
                    # ktl-outer: one V stationary load serves both chunks
                    for ktl, (koff, ksz) in enumerate(KEY_TILES):
                        pr = (prp[ktl // 2][:, ktl % 2, :] if ktl < 4
                              else prs[:, :])
                        for (qo, qw) in Q_CH:
                            o_sl = slice(qo, qo + qw)
                            nc.tensor.matmul(pv[:, o_sl],
                                             vp5[b][ktl][:ksz, vcol],
                                             pr[:ksz, o_sl],
                                             start=(ktl == 0),
                                             stop=(ktl == NKT - 1))
                    return pv

                def div_out(h, b, pv):
                    po = (h % 2) * 64
                    kt = h // 2
                    den = ep.tile([1, N], F32, tag="den", bufs=4, name="den")
                    nc.vector.tensor_copy(den[0:1, :], pv[64:65, :])
                    rec = ep.tile([1, N], F32, tag="den", bufs=4, name="rec")
                    nc.vector.reciprocal_approx_fast(out=rec, in_=den)
                    recb = ep.tile([64, N], F32, tag="recb", bufs=2,
                                   name="recb")
                    nc.gpsimd.partition_broadcast(recb[:], rec[0:1, :])
                    nc.vector.tensor_mul(
                        op[kt // 2][po:po + 64, kt % 2,
                                    b * N:(b + 1) * N],
                        pv[0:64, :], recb[:])

                def avdiv(hp, b, prz):
                    for i in range(2):
                        prp, prs = prz[i]
                        pv = attnv(2 * hp + i, b, prp, prs)
                        div_out(2 * hp + i, b, pv)

                pairs = [(hp, b) for hp in range(H // 2) for b in range(BL)]
                prevu = None
                for ui, u in enumerate(pairs):
                    pr = scores_probs_pair(*u, rewarm=(ui % 2 == 1))
                    if prevu is not None:
                        avdiv(prevu[0][0], prevu[0][1], prevu[1])
                    prevu = (u, pr)
                    hp, b = u
                    if b == 0 and hp + 1 < H // 2:
                        emit_qk(hp + 1, 0)
                        emit_qk(hp + 7, 1)
                avdiv(prevu[0][0], prevu[0][1], prevu[1])

                # ------------------------------------------------ proj (+res)
                for b in range(BL):
                    for mt in range(KT):
                        msl = slice(mt * 128, (mt + 1) * 128)
                        pss = []
                        for ci, (qo, qw) in enumerate(Q_CH):
                            ps = (psS(qw) if (mt + ci) % 2 == 0
                                  else psACC(qw))
                            pss.append((ps, qo, qw))
                        for t in range(KTP):
                            for ps, qo, qw in pss:
                                nc.tensor.matmul(
                                    ps, wproj_sb[t][:, :, msl],
                                    op[t][:, :, b * N + qo:b * N + qo + qw],
                                    start=(t == 0), stop=(t == KTP - 1),
                                    perf_mode=DR)
                        for ps, qo, qw in pss:
                            xs = x_sb[mt][:, b * N + qo:b * N + qo + qw]
                            nc.vector.affine_then_add(
                                xs, ps, xs, scale=cvec("g1", mt),
                                bias=cvec("g1bp", mt))

                # ------------------------------------- LN2 stats (+text n2)
                def n2_writer(kt, start, w, pair):
                    dst = n2_sb[kt]
                    if pair:
                        return p3(dst)[:, :, start:start + w]
                    return dst[:, start:start + w]

                # text tokens first so the text MLP can start early
                mn_t, r_t = ln_stats(x_sb, 0, TXT, True, psS, psACC)
                ln_elem(x_sb, n2_writer, 0, TXT, True, mn_t, r_t)
                # image stats now (psA still open); elementwise runs in the
                # MLP phase on vector/gpsimd, overlapped with the text MLP
                LN2I = [(TXT, 512, False), (N + TXT, 512, False),
                        (TXT + 512, 64, True)]
                ln2i_mr = [ln_stats(x_sb, o, w, pair, psS, psACC)
                           for (o, w, pair) in LN2I]

            # -------------------------------------------------- MLP phase
            with contextlib.ExitStack() as mctx:
                mp = mctx.enter_context(tc.tile_pool(name="mlp", bufs=1))
                psB = mctx.enter_context(
                    tc.tile_pool(name="psB", bufs=1, space="PSUM"))

                h_pair = [mp.tile([128, 2, NT], F8, tag="h", bufs=KT_HID // 2,
                                  name=f"h{i}") for i in range(KT_HID // 2)]

                wt1_sb = [mp.tile([128, HID], F8, tag="w1", bufs=KT,
                                  name=f"wt1_{i}") for i in range(KT)]
                for kt in range(KT):
                    nc.sync.dma_start(out=wt1_sb[kt][:],
                                      in_=wt1_d[kt * 128:(kt + 1) * 128, :])
                wt2_sb = [mp.tile([128, C], F8, tag="w2", bufs=KT_HID,
                                  name=f"wt2_{i}") for i in range(KT_HID)]
                for kt in range(KT_HID):
                    nc.sync.dma_start(out=wt2_sb[kt][:],
                                      in_=wt2_d[kt * 128:(kt + 1) * 128, :])

                def n2_writer2(kt, start, w, pair):
                    dst = n2_sb[kt]
                    if pair:
                        return p3(dst)[:, :, start:start + w]
                    return dst[:, start:start + w]

                # image-token LN2 elementwise (vector/gpsimd; overlaps the
                # text MLP running on PE/scalar)
                for (o, w, pair), (mn_, r_) in zip(LN2I, ln2i_mr):
                    ln_elem(x_sb, n2_writer2, o, w, pair, mn_, r_)

                def n2txt(kt):
                    return p3(n2_sb[kt])[:, :, 0:TXT]

                def htxt(kt):
                    return p3(h_pair[kt // 2][:, kt % 2, :])[:, :, 0:TXT]

                # ---- text mlp1 (plain fp8 matmuls, N=80 strided)
                for mt in range(KT_HID):
                    msl = slice(mt * 128, (mt + 1) * 128)
                    ps = psB.tile([128, 512], F32, tag="macc", bufs=8,
                                  name="m1ps")[:, :TXT * BL]
                    for kt in range(KT):
                        nc.tensor.matmul(ps, wt1_sb[kt][:, msl], n2txt(kt),
                                         start=(kt == 0), stop=(kt == KT - 1))
                    nc.scalar.activation(
                        htxt(mt), ps.rearrange("p (b n) -> p b n", b=BL),
                        AF.Gelu, bias=cvec("bt1", mt), scale=1.0 / 16)
                # ---- text mlp2
                for mt in range(KT):
                    msl = slice(mt * 128, (mt + 1) * 128)
                    ps = psB.tile([128, 512], F32, tag="macc", bufs=8,
                                  name="m2ps")[:, :TXT * BL]
                    for kt in range(KT_HID):
                        nc.tensor.matmul(ps, wt2_sb[kt][:, msl], htxt(kt),
                                         start=(kt == 0),
                                         stop=(kt == KT_HID - 1))
                    for b in range(BL):
                        xs = x_sb[mt][:, b * N:b * N + TXT]
                        nc.vector.affine_then_add(
                            xs, ps[:, b * TXT:(b + 1) * TXT], xs,
                            scale=cvec("g2", mt), bias=cvec("g2bt2", mt))

                # ---- image mlp (fp8 DoubleRow, K=256 per matmul)
                wi1_sb = [mp.tile([128, 2, HID], F8, tag="w1", bufs=KT,
                                  name=f"wi1_{i}") for i in range(3)]
                for t in range(3):
                    nc.sync.dma_start(out=wi1_sb[t][:], in_=wi1_d[t])
                wi2_sb = [mp.tile([128, 2, C], F8, tag="w2", bufs=KT_HID,
                                  name=f"wi2_{i}") for i in range(12)]
                for t in range(12):
                    nc.sync.dma_start(out=wi2_sb[t][:], in_=wi2_d[t])

                img_ch = []
                for b in range(BL):
                    img_ch += _chunks(N - TXT, 512, b * N + TXT)

                for mt in range(KT_HID):
                    msl = slice(mt * 128, (mt + 1) * 128)
                    pss = []
                    for (o, w) in img_ch:
                        ps = psB.tile([128, 512], F32, tag="macc", bufs=8,
                                      name="m1ip")[:, :w]
                        pss.append((ps, o, w))
                    # t-outer: one stationary load serves all four chunks
                    for t in range(3):
                        for ps, o, w in pss:
                            nc.tensor.matmul(ps, wi1_sb[t][:, :, msl],
                                             n2_pair[t][:, :, o:o + w],
                                             start=(t == 0), stop=(t == 2),
                                             perf_mode=DR)
                    for ps, o, w in pss:
                        nc.scalar.activation(
                            h_pair[mt // 2][:, mt % 2, o:o + w], ps, AF.Gelu,
                            bias=cvec("bi1", mt), scale=1.0 / 16)
                for mt in range(KT):
                    msl = slice(mt * 128, (mt + 1) * 128)
                    pss = []
                    for (o, w) in img_ch:
                        ps = psB.tile([128, 512], F32, tag="macc", bufs=8,
                                      name="m2ip")[:, :w]
                        pss.append((ps, o, w))
                    for t in range(12):
                        for ps, o, w in pss:
                            nc.tensor.matmul(ps, wi2_sb[t][:, :, msl],
                                             h_pair[t][:, :, o:o + w],
                                             start=(t == 0), stop=(t == 11),
                                             perf_mode=DR)
                    for ps, o, w in pss:
                        xs = x_sb[mt][:, o:o + w]
                        nc.vector.affine_then_add(
                            xs, ps, xs, scale=cvec("g2", mt),
                            bias=cvec("g2bi2", mt))
                    # this channel tile of x is final: store it
                    for b in range(BL):
                        nc.sync.dma_start(
                            out=out_d[b, mt * 128:(mt + 1) * 128, :],
                            in_=x_sb[mt][:, b * N:(b + 1) * N])

    nc.compile()
    return nc


# ---------------------------------------------------------- host-side prep
def _prep_inputs(inputs):
    f = lambda k: np.asarray(inputs[k], dtype=np.float32)
    x = f("x")
    rel_bias = f("rel_bias")
    w_qkv = f("w_qkv")
    ln1_g, ln1_b = f("ln1_g"), f("ln1_b")
    q_bias, v_bias = f("q_bias"), f("v_bias")
    w_proj, b_proj = f("w_proj"), f("b_proj")
    gamma1, gamma2 = f("gamma1"), f("gamma2")

    Wq = w_qkv[:C] * ln1_g[None, :]
    Wk = w_qkv[C:2 * C] * ln1_g[None, :]
    Wv = w_qkv[2 * C:] * ln1_g[None, :]
    bq = q_bias + w_qkv[:C] @ ln1_b
    bk = w_qkv[C:2 * C] @ ln1_b
    bv = v_bias + w_qkv[2 * C:] @ ln1_b
    Wq *= SCALE
    bq *= SCALE
    b_projp = b_proj + w_proj @ bv

    wqk = np.concatenate([Wq, Wk], axis=0).T          # [C, 1536]
    wqk_s = wqk.copy()
    wqk_s[:, :C] *= SW_Q
    wqk_s[:, C:] *= SW_K
    wv = Wv.T * SW_V                                  # [C, C]
    wproj = w_proj.T * SW_P                           # [C, C]

    def mlp_fold(w1, b1, w2, b2, g, bb):
        w1f = w1 * g[None, :]
        b1f = b1 + w1 @ bb
        return w1f.T, b1f, w2.T, b2

    wt1, bt1, wt2, bt2 = mlp_fold(f("wt1"), f("bt1"), f("wt2"), f("bt2"),
                                  f("ln2t_g"), f("ln2t_b"))
    wi1, bi1, wi2, bi2 = mlp_fold(f("wi1"), f("bi1"), f("wi2"), f("bi2"),
                                  f("ln2i_g"), f("ln2i_b"))

    gamma1_s = gamma1 / SW_P
    gamma2 = gamma2 / 16.0  # mlp weights are pre-scaled by 16 for fp8
    consts = np.zeros((128, NCONST), np.float32)
    bqk = np.concatenate([bq, bk])
    for i in range(MT_QK):
        consts[:, CCOL["bqk"] + i] = bqk[i * 128:(i + 1) * 128]
    for i in range(KT):
        sl = slice(i * 128, (i + 1) * 128)
        consts[:, CCOL["g1"] + i] = gamma1_s[sl]
        consts[:, CCOL["g1bp"] + i] = (gamma1 * b_projp)[sl]
        consts[:, CCOL["g2"] + i] = gamma2[sl]
        consts[:, CCOL["g2bt2"] + i] = (16.0 * gamma2 * bt2)[sl]
        consts[:, CCOL["g2bi2"] + i] = (16.0 * gamma2 * bi2)[sl]
    for i in range(KT_HID):
        sl = slice(i * 128, (i + 1) * 128)
        consts[:, CCOL["bt1"] + i] = bt1[sl]
        consts[:, CCOL["bi1"] + i] = bi1[sl]

    # raw rel_bias transposed to [H, key, query], keys padded to 640
    ebt = rel_bias.transpose(0, 2, 1)
    eb = np.zeros((H, NKT * 128, N), np.float32)
    eb[:, :N, :] = ebt
    eb = eb.reshape(H, NKT, 128, N)

    bf = lambda a: np.ascontiguousarray(a, dtype=np.float32).astype(bfloat16)
    f8r = lambda a: np.ascontiguousarray(a).astype(ml_dtypes.float8_e4m3)
    f8x = lambda a: np.ascontiguousarray(
        np.asarray(a, dtype=np.float32) * 16.0).astype(ml_dtypes.float8_e4m3)

    def drpair(a, npair, width):           # [C, width] -> [npair,128,2,width]
        return np.ascontiguousarray(
            f8r(a).reshape(npair, 2, 128, width).transpose(0, 2, 1, 3))

    wi1_dr = f8x(wi1).reshape(3, 2, 128, HID).transpose(0, 2, 1, 3)
    wi2_dr = f8x(wi2).reshape(12, 2, 128, C).transpose(0, 2, 1, 3)
    shared = {
        "relbias": bf(eb),
        "id128": np.ascontiguousarray(np.eye(128, dtype=np.float32)
                                      ).astype(bfloat16),
        "wqk": drpair(wqk_s, KTP, 2 * C),
        "wv": drpair(wv, KTP, C),
        "wproj": drpair(wproj, KTP, C),
        "wt1": f8x(wt1), "wi1": np.ascontiguousarray(wi1_dr),
        "wt2": f8x(wt2), "wi2": np.ascontiguousarray(wi2_dr),
        "consts": np.ascontiguousarray(consts),
    }
    # per-core x shards, channel-major
    xs = x.reshape(NCORES, BL, N, C).transpose(0, 1, 3, 2)
    in_maps = []
    for c in range(NCORES):
        m = dict(shared)
        m["x"] = np.ascontiguousarray(xs[c])
        in_maps.append(m)
    return in_maps


def kernel(**inputs):
    global LAST_EXEC_TIME_NS, LAST_TRACE_PATH
    _install_ntff_hook()
    from concourse.bass_utils import run_bass_kernel_spmd

    if "nc" not in _NC_CACHE:
        _NC_CACHE["nc"] = _build_nc()
    nc = _NC_CACHE["nc"]

    in_maps = _prep_inputs(inputs)
    trace = os.environ.get(_ENV_TRACE, "") == "1"
    import time as _time
    if LAST_EXEC_TIME_NS is not None and trace:
        _time.sleep(2.0)  # let the previous NRT profile session settle
    res = run_bass_kernel_spmd(nc, in_maps, core_ids=list(range(NCORES)),
                               trace=trace)
    LAST_EXEC_TIME_NS = res.exec_time_ns
    if trace and res.instructions_and_trace is not None:
        LAST_TRACE_PATH = res.instructions_and_trace[1]

    out = np.empty((B, N, C), np.float32)
    for c in range(NCORES):
        oc = np.asarray(res.results[c]["out"])          # [BL, C, N]
        out[c * BL:(c + 1) * BL] = oc.transpose(0, 2, 1)
    return out



# revision 6
# speedup vs baseline: 1.3259x; 1.3259x over previous
"""Trainium2 Bass kernel for nn_Block_55448027791422 (dense transformer block).

Strategy: pure data-parallel over batch B=16 across 8 NeuronCores (2 batches
per core), zero collectives.  Activations live on-chip channel-major (C on
partitions, tokens on free dim).  v3 changes vs v2:
  - residual stream x lives in SBUF as bf16 (halves x/out DMA, removes the
    per-chunk bf16 staging copies the LN stats matmuls needed).
  - attn@V really runs fp8 DoubleRow over key-tile pairs (V is stored as
    [128, 2, H*VHW] pair tiles); only the 104-key tail tile runs plain.
  - softmax denominator reciprocal reads the psum row directly (no staging
    copy); V-tile memsets touch only the denominator ones-columns.
  - q/k psum->sbuf copies moved from Scalar to Vector (tensor_scalar with
    per-partition bias), freeing Scalar for the exp stream.
  - image-MLP matmuls write one 3-bank [128, 1536] psum tile per output
    tile (bank-aligned chunks), so GELU / the residual affine run as ONE
    instruction per tile instead of four (Scalar was the mlp1 pacer).
  - HAM rewarm bursts removed (PE stream is dense enough to stay warm).
"""

import os
import sys
import contextlib
import ctypes
import types

import numpy as np

for _p in ("/opt/trn_rl_repo",):
    if _p not in sys.path:
        sys.path.insert(0, _p)

import ml_dtypes

bfloat16 = ml_dtypes.bfloat16

# ---------------------------------------------------------------- constants
B, N, C, H, HD, HID, TXT = 16, 616, 768, 12, 64, 3072, 40
NCORES = 8
BL = B // NCORES            # 2 batches per core
NT = BL * N                 # 1232 tokens per core
KT = C // 128               # 6 channel tiles
KTP = KT // 2               # 3 channel-pair tiles (fp8 DoubleRow)
MT_QK = (2 * C) // 128      # 12 output tiles for q,k
KT_HID = HID // 128         # 24 hidden tiles
EPS = 1e-5
KEY_TILES = [(0, 128), (128, 128), (256, 128), (384, 128), (512, 104)]
NKT = len(KEY_TILES)
SCALE = HD ** -0.5
SW_Q, SW_K, SW_V, SW_P = 128.0, 16.0, 16.0, 16.0   # fp8 weight pre-scales
EXP_SHIFT = 6.5             # probs = exp(score - EXP_SHIFT); cancels in ratio
VHW = 80                    # per-head column pitch in the V tiles (65 used)

# image-token matmul chunks: (token offset, width, psum offset) -- psum
# offsets are chosen so every chunk stays inside one 512-fp32 psum bank and
# the 1152 image columns are contiguous as [2, 576] for one-shot GELU/affine.
IMG_CH4 = [(TXT, 512, 0), (TXT + 512, 64, 512),
           (N + TXT, 448, 576), (N + TXT + 448, 128, 1024)]


def _chunks(total, step=512, base=0):
    out, o = [], 0
    while o < total:
        s = min(step, total - o)
        out.append((base + o, s))
        o += s
    return out


NT_CH = _chunks(NT)                      # [(0,512),(512,512),(1024,208)]
Q_CH = [(0, 512), (512, 104)]            # per-batch query chunks


# const-vector column indexes in the [128, NCONST] consts tensor
def _cc():
    idx = {}
    c = 0
    for name, n in [("bqk", MT_QK), ("g1", KT), ("g1bp", KT),
                    ("g2", KT), ("g2bt2", KT), ("g2bi2", KT),
                    ("bt1", KT_HID), ("bi1", KT_HID)]:
        idx[name] = c
        c += n
    return idx, c


CCOL, NCONST = _cc()

_ENV_TRACE = "BASS_KERNEL_TRACE"
LAST_EXEC_TIME_NS = None
LAST_TRACE_PATH = None


# ------------------------------------------------------- axon profile hook
def _install_ntff_hook():
    """run_bass_kernel_spmd(trace=True) under axon needs antenv.axon_hooks."""
    if "antenv.axon_hooks" in sys.modules:
        return
    so_path = "/opt/axon/libaxon_pjrt.so"
    state = {"h": None}

    def _build():
        try:
            lib = ctypes.CDLL(so_path)
        except OSError:
            return None
        if not hasattr(lib, "axon_start_nrt_profile"):
            return None
        lib.axon_start_nrt_profile.argtypes = [ctypes.POINTER(ctypes.c_int64),
                                               ctypes.c_size_t]
        lib.axon_start_nrt_profile.restype = ctypes.c_int64
        lib.axon_stop_nrt_profile.argtypes = [ctypes.c_char_p]
        lib.axon_stop_nrt_profile.restype = ctypes.c_int64

        @contextlib.contextmanager
        def _hook(output_dir, device_ids):
            import jax
            jax.devices()
            if device_ids:
                ids = (ctypes.c_int64 * len(device_ids))(*device_ids)
                rc = lib.axon_start_nrt_profile(ids, len(device_ids))
            else:
                rc = lib.axon_start_nrt_profile(None, 0)
            if rc != 0:
                raise RuntimeError(f"axon_start_nrt_profile rc={rc}")
            try:
                yield
            finally:
                n = lib.axon_stop_nrt_profile(str(output_dir).encode())
                if n < 0:
                    raise RuntimeError(f"axon_stop_nrt_profile rc={n}")

        return _hook

    def get_axon_ntff_profile_hook():
        if state["h"] is None:
            state["h"] = _build()
        return state["h"]

    mod = types.ModuleType("antenv.axon_hooks")
    mod.get_axon_ntff_profile_hook = get_axon_ntff_profile_hook
    mod.set_axon_ntff_profile_hook = lambda h: state.update(h=h)
    sys.modules["antenv.axon_hooks"] = mod


# ------------------------------------------------------------ graph builder
_NC_CACHE = {}


def _build_nc():
    import concourse.bass as bass  # noqa: F401
    import concourse.mybir as mybir
    import concourse.tile as tile
    from concourse import bacc

    F32 = mybir.dt.float32
    BF16 = mybir.dt.bfloat16
    F8 = mybir.dt.float8e4
    AF = mybir.ActivationFunctionType
    ALU = mybir.AluOpType
    DR = mybir.MatmulPerfMode.DoubleRow

    nc = bacc.Bacc(None, target_bir_lowering=False)
    d = nc.declare_dram_parameter
    x_d = d("x", [BL, C, N], BF16, isOutput=False)
    eb_d = d("relbias", [H, NKT, 128, N], BF16, isOutput=False)
    wqk_d = d("wqk", [KTP, 128, 2, 2 * C], F8, isOutput=False)
    wv_d = d("wv", [KTP, 128, 2, C], F8, isOutput=False)
    wproj_d = d("wproj", [KTP, 128, 2, C], F8, isOutput=False)
    id_d = d("id128", [128, 128], BF16, isOutput=False)
    wt1_d = d("wt1", [C, HID], F8, isOutput=False)
    wi1_d = d("wi1", [3, 128, 2, HID], F8, isOutput=False)
    wt2_d = d("wt2", [HID, C], F8, isOutput=False)
    wi2_d = d("wi2", [12, 128, 2, C], F8, isOutput=False)
    consts_d = d("consts", [128, NCONST], F32, isOutput=False)
    out_d = d("out", [BL, C, N], BF16, isOutput=True)

    def p3(t):
        """[128, NT] view -> [128, BL, N] batch-split view."""
        return t.rearrange("p (b n) -> p b n", b=BL)

    with tile.TileContext(nc) as tc:
        with contextlib.ExitStack() as octx:
            per = octx.enter_context(tc.tile_pool(name="perm", bufs=1))
            # persistent tiles
            consts = per.tile([128, NCONST], F32, tag="consts")

            def cvec(name, i):
                return consts[:, CCOL[name] + i:CCOL[name] + i + 1]

            nc.sync.dma_start(out=consts[:], in_=consts_d[:])
            ones128 = per.tile([128, 128], BF16, tag="ones128")
            nc.vector.memset(ones128[:], 1.0)
            id128 = per.tile([128, 128], BF16, tag="id128")
            nc.sync.dma_start(out=id128[:], in_=id_d[:])
            eps_ap = per.tile([128, 1], F32, tag="epsap")
            nc.vector.memset(eps_ap[:], EPS)
            shift_ap = per.tile([128, 1], F32, tag="shiftap")
            nc.vector.memset(shift_ap[:], -float(EXP_SHIFT))

            x_sb = [per.tile([128, NT], BF16, tag="x", bufs=KT, name=f"x{i}")
                    for i in range(KT)]
            for kt in range(KT):
                for b in range(BL):
                    nc.sync.dma_start(
                        out=x_sb[kt][:, b * N:(b + 1) * N],
                        in_=x_d[b, kt * 128:(kt + 1) * 128, :])

            # fp8 channel-pair activation tiles (DoubleRow operands):
            # pair t holds channel tiles (2t, 2t+1) in slots (0, 1)
            n1p = [per.tile([128, 2, NT], F8, tag="n1p", bufs=KTP,
                            name=f"n1p{i}") for i in range(KTP)]
            op = [per.tile([128, 2, NT], F8, tag="op", bufs=KTP,
                           name=f"op{i}") for i in range(KTP)]
            # V tiles: pair tiles over key tiles (0,1) and (2,3) for fp8
            # DoubleRow attn@V, plus the plain 104-key tail tile.  Per-head
            # pitch VHW; col h*VHW+64 holds the softmax-denominator ones.
            vpP = [[per.tile([128, 2, H * VHW], F8, tag="vpp", bufs=2 * BL,
                             name=f"vpp{b}_{tp}") for tp in range(2)]
                   for b in range(BL)]
            vp4 = [per.tile([128, H * VHW], F8, tag="vp4", bufs=BL,
                            name=f"vp4{b}") for b in range(BL)]

            n2_pair = [per.tile([128, 2, NT], F8, tag="n2p", bufs=KTP,
                                name=f"n2p{i}") for i in range(KTP)]
            n2_sb = [n2_pair[i // 2][:, i % 2, :] for i in range(KT)]

            # LN scratch lives in the outer pool (used by both phases)
            def ln_scratch(tag, w, dt, bufs, name):
                return per.tile([128, 512], dt, tag=tag, bufs=bufs,
                                name=name)[:, :w]

            # ------------------------------------------------ LayerNorm
            def ln_stats(x_tiles, start, w, pair, psm_fn, pse_fn):
                """Stats for one token chunk -> (mn, r) [128, fw] tiles.
                pair=True: b-symmetric range [start,start+w) of BOTH batches
                (within-batch offset), free size 2w.  Else NT-offset."""
                fw = 2 * w if pair else w

                def src(kt):
                    if pair:
                        return p3(x_tiles[kt])[:, :, start:start + w]
                    return x_tiles[kt][:, start:start + w]

                ps_m = psm_fn(fw)
                ps_e = pse_fn(fw)
                for kt in range(KT):
                    sqt = ln_scratch("sqt", fw, BF16, 3, "sqt")
                    sqt_v = (sqt.rearrange("p (b n) -> p b n", b=2)
                             if pair else sqt)
                    if kt % 2 == 0:
                        nc.scalar.activation(sqt_v, src(kt), AF.Square)
                    else:
                        nc.gpsimd.tensor_mul(sqt_v, src(kt), src(kt))
                    nc.tensor.matmul(ps_m, ones128[:], src(kt),
                                     start=(kt == 0), stop=(kt == KT - 1))
                    nc.tensor.matmul(ps_e, ones128[:], sqt,
                                     start=(kt == 0), stop=(kt == KT - 1))
                m2 = ln_scratch("lntmp", fw, F32, 3, "m2")
                nc.scalar.activation(m2, ps_m, AF.Square,
                                     scale=float(C ** -0.5))
                dd = ln_scratch("lntmp", fw, F32, 3, "dd")
                nc.vector.tensor_sub(dd, ps_e, m2)
                s = ln_scratch("lntmp", fw, F32, 3, "s")
                nc.scalar.activation(s, dd, AF.Sqrt, bias=eps_ap[:, 0:1],
                                     scale=float(1.0 / C))
                r = ln_scratch("lnr", fw, F32, 8, "r")
                nc.vector.reciprocal_approx_fast(out=r, in_=s)
                mn = ln_scratch("lnr", fw, F32, 8, "mn")
                nc.scalar.mul(mn, ps_m, float(1.0 / C))
                return mn, r

            def ln_elem(x_tiles, out_writer, start, w, pair, mn, r):
                fw = 2 * w if pair else w

                def src(kt):
                    if pair:
                        return p3(x_tiles[kt])[:, :, start:start + w]
                    return x_tiles[kt][:, start:start + w]

                mn_v = (mn.rearrange("p (b n) -> p b n", b=2)
                        if pair else mn)
                r_v = r.rearrange("p (b n) -> p b n", b=2) if pair else r
                for kt in range(KT):
                    t = ln_scratch("lnt", fw, BF16, 4, "t")
                    tv = (t.rearrange("p (b n) -> p b n", b=2)
                          if pair else t)
                    e1, e2 = ((nc.gpsimd, nc.vector) if kt % 2 == 0
                              else (nc.vector, nc.gpsimd))
                    e1.tensor_sub(tv, src(kt), mn_v)
                    dst = out_writer(kt, start, w, pair)
                    e2.tensor_mul(dst, tv, r_v)

            # ---------------- early pool (qkv / attention / proj / stats)
            with contextlib.ExitStack() as ectx:
                ep = ectx.enter_context(tc.tile_pool(name="early", bufs=1))
                psA = ectx.enter_context(
                    tc.tile_pool(name="psA", bufs=1, space="PSUM"))

                def psS(w=616):
                    return psA.tile([128, 616], F32, tag="S", bufs=2,
                                    name="psS")[:, :w]

                def psACC(w=616):
                    return psA.tile([128, 616], F32, tag="acc", bufs=2,
                                    name="psAcc")[:, :w]

                # early-dying tags first so the mlp pool can reuse space
                wqk_sb = [ep.tile([128, 2, 2 * C], F8, tag="wqk", bufs=KTP,
                                  name=f"wqk{i}") for i in range(KTP)]
                _d2 = ep.tile([128, 2, 624], F8, tag="prp", bufs=8, name="d2")
                _d3 = ep.tile([128, 624], F8, tag="prs", bufs=4, name="d3")
                _d4 = ep.tile([128, N], BF16, tag="eb", bufs=15, name="d4")
                _d5 = ep.tile([1, N], F32, tag="den", bufs=4, name="d5")
                _d6 = ep.tile([64, N], F32, tag="recb", bufs=2, name="d6")
                wv_sb = [ep.tile([128, 2, C], F8, tag="wv", bufs=KTP,
                                 name=f"wv{i}") for i in range(KTP)]
                wproj_sb = [ep.tile([128, 2, C], F8, tag="wproj", bufs=KTP,
                                    name=f"wpj{i}") for i in range(KTP)]
                for t in range(KTP):
                    nc.sync.dma_start(out=wqk_sb[t][:], in_=wqk_d[t])
                    nc.sync.dma_start(out=wv_sb[t][:], in_=wv_d[t])
                for t in range(KTP):
                    nc.sync.dma_start(out=wproj_sb[t][:], in_=wproj_d[t])

                qk_sb = [ep.tile([128, NT], BF16, tag="qk", bufs=MT_QK,
                                 name=f"qk{i}") for i in range(MT_QK)]

                # -------------------------------------------------- LN1
                def n1_writer(kt, start, w, pair):
                    dst = n1p[kt // 2][:, kt % 2, :]
                    if pair:
                        return p3(dst)[:, :, start:start + w]
                    return dst[:, start:start + w]

                for (o, w) in NT_CH:
                    mn, r = ln_stats(x_sb, o, w, False, psS, psACC)
                    ln_elem(x_sb, n1_writer, o, w, False, mn, r)

                # ------------------------------------------------- v matmul
                # ones columns for the softmax denominator (only cols used
                # by the attn@V stationary reads need initialising)
                for b in range(BL):
                    for tp in range(2):
                        for s in range(2):
                            v3s = (vpP[b][tp][:, s, :]
                                   .rearrange("p (h e) -> p h e", e=VHW))
                            nc.vector.memset(v3s[:, :, 64:65], 1.0)
                    v3 = vp4[b].rearrange("p (h e) -> p h e", e=VHW)
                    nc.vector.memset(v3[:104, :, 64:65], 1.0)
                for b in range(BL):
                    for ktl, (koff, ksz) in enumerate(KEY_TILES):
                        toff = b * N + koff
                        if ktl < 4:
                            vt3 = (vpP[b][ktl // 2][:, ktl % 2, :]
                                   .rearrange("p (h e) -> p h e", e=VHW))
                        else:
                            vt3 = vp4[b].rearrange("p (h e) -> p h e", e=VHW)
                        vch = [(0, 512), (512, 256)]
                        pss = [(psS(512) if ci == 0 else psACC(256))[:ksz, :]
                               for ci in range(2)]
                        # t-outer: one stationary load serves both chunks
                        for t in range(KTP):
                            for ci, (o, w) in enumerate(vch):
                                nc.tensor.matmul(
                                    pss[ci], n1p[t][:, :, toff:toff + ksz],
                                    wv_sb[t][:, :, o:o + w],
                                    start=(t == 0), stop=(t == KTP - 1),
                                    perf_mode=DR)
                        for ci, (o, w) in enumerate(vch):
                            nheads = w // 64
                            h0 = o // 64
                            nc.scalar.activation(
                                vt3[:ksz, h0:h0 + nheads, 0:64],
                                pss[ci].rearrange("p (h e) -> p h e", e=64),
                                AF.Copy, scale=float(1.0 / SW_V))

                # ------------------------------------------------ q,k matmul
                def emit_qk(mt, mi):
                    msl = slice(mt * 128, (mt + 1) * 128)
                    sw = SW_Q if mt < KT else SW_K
                    pss = []
                    for ci, (o, w) in enumerate(NT_CH):
                        ps = (psS(w) if (mi + ci) % 2 == 0 else psACC(w))
                        pss.append((ps, o, w))
                    # t-outer: one stationary load serves all three chunks
                    for t in range(KTP):
                        for ps, o, w in pss:
                            nc.tensor.matmul(ps, wqk_sb[t][:, :, msl],
                                             n1p[t][:, :, o:o + w],
                                             start=(t == 0),
                                             stop=(t == KTP - 1),
                                             perf_mode=DR)
                    for ps, o, w in pss:
                        nc.scalar.activation(qk_sb[mt][:, o:o + w], ps,
                                             AF.Identity,
                                             bias=cvec("bqk", mt),
                                             scale=float(1.0 / sw))

                # only the first head-pair's q/k upfront; the rest are
                # emitted inside the attention loop one pair ahead, so the
                # qkv matmuls fill attention's exp-bound PE stalls
                emit_qk(0, 0)
                emit_qk(6, 1)

                # ------------------------------------------------ attention
                eb_pool = [None] * (H * NKT)

                def eb_tile(h, ktl):
                    i = h * NKT + ktl
                    if eb_pool[i] is None:
                        t = ep.tile([128, N], BF16, tag="eb", bufs=15,
                                    name=f"eb{i}")
                        nc.sync.dma_start(out=t[:], in_=eb_d[h, ktl])
                        eb_pool[i] = t
                    return eb_pool[i]

                def scores_probs_pair(hp, b):
                    """Scores for heads (2hp, 2hp+1) of batch b.  The two
                    heads' q/k live in rows 0-63 / 64-127 of the same tiles,
                    so their qk matmuls land in disjoint PE row groups and
                    run concurrently when issued back-to-back.  psum =
                    rel_bias (identity matmul) + q.k; praw = exp(s-shift)
                    as fp8 pairs, one set per head."""
                    qt = qk_sb[hp]
                    kt_t = qk_sb[KT + hp]
                    prz = []
                    for _ in range(2):
                        prp = [ep.tile([128, 2, 624], F8, tag="prp", bufs=8,
                                       name="prp") for _ in range(2)]
                        prs = ep.tile([128, 624], F8, tag="prs", bufs=4,
                                      name="prs")
                        prz.append((prp, prs))
                    for ktl, (koff, ksz) in enumerate(KEY_TILES):
                        pz = [psS(), psS()]
                        ebz = [eb_tile(2 * hp, ktl),
                               eb_tile(2 * hp + 1, ktl)]
                        ksl = slice(b * N + koff, b * N + koff + ksz)
                        for i in range(2):
                            for (qo, qw) in Q_CH:
                                nc.tensor.matmul(
                                    pz[i][:ksz, qo:qo + qw], id128[:, :ksz],
                                    ebz[i][:, qo:qo + qw],
                                    start=True, stop=False)
                        # adjacent qk matmuls: rows 0-63 then 64-127
                        for i in range(2):
                            po = i * 64
                            for (qo, qw) in Q_CH:
                                nc.tensor.matmul(
                                    pz[i][:ksz, qo:qo + qw],
                                    kt_t[po:po + 64, ksl],
                                    qt[po:po + 64,
                                       b * N + qo:b * N + qo + qw],
                                    start=False, stop=True)
                        for i in range(2):
                            prp, prs = prz[i]
                            dst = (prp[ktl // 2][:, ktl % 2, :N] if ktl < 4
                                   else prs[:, :N])
                            nc.scalar.activation(dst[:ksz, :],
                                                 pz[i][:ksz, :], AF.Exp,
                                                 bias=shift_ap[:ksz, 0:1])
                    return prz

                def attnv(h, b, prp, prs):
                    vcol = slice(h * VHW, h * VHW + 65)
                    pv = psACC()[:65, :]
                    # fp8 DoubleRow over key-tile pairs; 104-key tail plain
                    for tp in range(2):
                        for (qo, qw) in Q_CH:
                            nc.tensor.matmul(pv[:, qo:qo + qw],
                                             vpP[b][tp][:, :, vcol],
                                             prp[tp][:, :, qo:qo + qw],
                                             start=(tp == 0), stop=False,
                                             perf_mode=DR)
                    for (qo, qw) in Q_CH:
                        nc.tensor.matmul(pv[:, qo:qo + qw],
                                         vp4[b][:104, vcol],
                                         prs[:104, qo:qo + qw],
                                         start=False, stop=True)
                    return pv

                def div_out(h, b, pv):
                    po = (h % 2) * 64
                    kt = h // 2
                    den = ep.tile([1, N], F32, tag="den", bufs=4, name="den")
                    nc.vector.tensor_copy(den[0:1, :], pv[64:65, :])
                    rec = ep.tile([1, N], F32, tag="den", bufs=4, name="rec")
                    nc.vector.reciprocal_approx_fast(out=rec, in_=den)
                    recb = ep.tile([64, N], F32, tag="recb", bufs=2,
                                   name="recb")
                    nc.gpsimd.partition_broadcast(recb[:], rec[0:1, :])
                    nc.vector.tensor_mul(
                        op[kt // 2][po:po + 64, kt % 2,
                                    b * N:(b + 1) * N],
                        pv[0:64, :], recb[:])

                def avdiv(hp, b, prz):
                    for i in range(2):
                        prp, prs = prz[i]
                        pv = attnv(2 * hp + i, b, prp, prs)
                        div_out(2 * hp + i, b, pv)

                pairs = [(hp, b) for hp in range(H // 2) for b in range(BL)]
                prevu = None
                for ui, u in enumerate(pairs):
                    pr = scores_probs_pair(*u)
                    if prevu is not None:
                        avdiv(prevu[0][0], prevu[0][1], prevu[1])
                    prevu = (u, pr)
                    hp, b = u
                    if b == 0 and hp + 1 < H // 2:
                        emit_qk(hp + 1, 0)
                        emit_qk(hp + 7, 1)
                avdiv(prevu[0][0], prevu[0][1], prevu[1])

                # ------------------------------------------------ proj (+res)
                for b in range(BL):
                    for mt in range(KT):
                        msl = slice(mt * 128, (mt + 1) * 128)
                        pss = []
                        for ci, (qo, qw) in enumerate(Q_CH):
                            ps = (psS(qw) if (mt + ci) % 2 == 0
                                  else psACC(qw))
                            pss.append((ps, qo, qw))
                        for t in range(KTP):
                            for ps, qo, qw in pss:
                                nc.tensor.matmul(
                                    ps, wproj_sb[t][:, :, msl],
                                    op[t][:, :, b * N + qo:b * N + qo + qw],
                                    start=(t == 0), stop=(t == KTP - 1),
                                    perf_mode=DR)
                        for ps, qo, qw in pss:
                            xs = x_sb[mt][:, b * N + qo:b * N + qo + qw]
                            nc.vector.affine_then_add(
                                xs, ps, xs, scale=cvec("g1", mt),
                                bias=cvec("g1bp", mt))

                # ------------------------------------- LN2 stats (+text n2)
                def n2_writer(kt, start, w, pair):
                    dst = n2_sb[kt]
                    if pair:
                        return p3(dst)[:, :, start:start + w]
                    return dst[:, start:start + w]

                # text tokens first so the text MLP can start early
                mn_t, r_t = ln_stats(x_sb, 0, TXT, True, psS, psACC)
                ln_elem(x_sb, n2_writer, 0, TXT, True, mn_t, r_t)
                # image stats now (psA still open); elementwise runs in the
                # MLP phase on vector/gpsimd, overlapped with the text MLP
                LN2I = [(TXT, 512, False), (N + TXT, 512, False),
                        (TXT + 512, 64, True)]
                ln2i_mr = [ln_stats(x_sb, o, w, pair, psS, psACC)
                           for (o, w, pair) in LN2I]

            # -------------------------------------------------- MLP phase
            with contextlib.ExitStack() as mctx:
                mp = mctx.enter_context(tc.tile_pool(name="mlp", bufs=1))
                psB = mctx.enter_context(
                    tc.tile_pool(name="psB", bufs=1, space="PSUM"))

                h_pair = [mp.tile([128, 2, NT], F8, tag="h", bufs=KT_HID // 2,
                                  name=f"h{i}") for i in range(KT_HID // 2)]

                wt1_sb = [mp.tile([128, HID], F8, tag="w1", bufs=KT,
                                  name=f"wt1_{i}") for i in range(KT)]
                for kt in range(KT):
                    nc.sync.dma_start(out=wt1_sb[kt][:],
                                      in_=wt1_d[kt * 128:(kt + 1) * 128, :])
                wt2_sb = [mp.tile([128, C], F8, tag="w2", bufs=KT_HID,
                                  name=f"wt2_{i}") for i in range(KT_HID)]
                for kt in range(KT_HID):
                    nc.sync.dma_start(out=wt2_sb[kt][:],
                                      in_=wt2_d[kt * 128:(kt + 1) * 128, :])

                def n2_writer2(kt, start, w, pair):
                    dst = n2_sb[kt]
                    if pair:
                        return p3(dst)[:, :, start:start + w]
                    return dst[:, start:start + w]

                # image-token LN2 elementwise (vector/gpsimd; overlaps the
                # text MLP running on PE/scalar)
                for (o, w, pair), (mn_, r_) in zip(LN2I, ln2i_mr):
                    ln_elem(x_sb, n2_writer2, o, w, pair, mn_, r_)

                def n2txt(kt):
                    return p3(n2_sb[kt])[:, :, 0:TXT]

                def htxt(kt):
                    return p3(h_pair[kt // 2][:, kt % 2, :])[:, :, 0:TXT]

                # ---- text mlp1 (plain fp8 matmuls, N=80 strided)
                for mt in range(KT_HID):
                    msl = slice(mt * 128, (mt + 1) * 128)
                    ps = psB.tile([128, 512], F32, tag="tacc", bufs=2,
                                  name="m1ps")[:, :TXT * BL]
                    for kt in range(KT):
                        nc.tensor.matmul(ps, wt1_sb[kt][:, msl], n2txt(kt),
                                         start=(kt == 0), stop=(kt == KT - 1))
                    nc.scalar.activation(
                        htxt(mt), ps.rearrange("p (b n) -> p b n", b=BL),
                        AF.Gelu, bias=cvec("bt1", mt), scale=1.0 / 16)
                # ---- text mlp2
                for mt in range(KT):
                    msl = slice(mt * 128, (mt + 1) * 128)
                    ps = psB.tile([128, 512], F32, tag="tacc", bufs=2,
                                  name="m2ps")[:, :TXT * BL]
                    for kt in range(KT_HID):
                        nc.tensor.matmul(ps, wt2_sb[kt][:, msl], htxt(kt),
                                         start=(kt == 0),
                                         stop=(kt == KT_HID - 1))
                    for b in range(BL):
                        xs = x_sb[mt][:, b * N:b * N + TXT]
                        nc.vector.affine_then_add(
                            xs, ps[:, b * TXT:(b + 1) * TXT], xs,
                            scale=cvec("g2", mt), bias=cvec("g2bt2", mt))

                # ---- image mlp (fp8 DoubleRow, K=256 per matmul; one
                # 3-bank psum tile per output tile -> single GELU/affine)
                wi1_sb = [mp.tile([128, 2, HID], F8, tag="w1", bufs=KT,
                                  name=f"wi1_{i}") for i in range(3)]
                for t in range(3):
                    nc.sync.dma_start(out=wi1_sb[t][:], in_=wi1_d[t])
                wi2_sb = [mp.tile([128, 2, C], F8, tag="w2", bufs=KT_HID,
                                  name=f"wi2_{i}") for i in range(12)]
                for t in range(12):
                    nc.sync.dma_start(out=wi2_sb[t][:], in_=wi2_d[t])

                for mt in range(KT_HID):
                    msl = slice(mt * 128, (mt + 1) * 128)
                    ps = psB.tile([128, 1536], F32, tag="macc", bufs=2,
                                  name="m1ip")
                    for t in range(3):
                        for (o, w, po) in IMG_CH4:
                            nc.tensor.matmul(ps[:, po:po + w],
                                             wi1_sb[t][:, :, msl],
                                             n2_pair[t][:, :, o:o + w],
                                             start=(t == 0), stop=(t == 2),
                                             perf_mode=DR)
                    ps3 = ps[:, :1152].rearrange("p (b n) -> p b n", b=2)
                    hs = (h_pair[mt // 2][:, mt % 2, :]
                          .rearrange("p (b n) -> p b n", b=2)[:, :, TXT:])
                    nc.scalar.activation(hs, ps3, AF.Gelu,
                                         bias=cvec("bi1", mt), scale=1.0 / 16)
                for mt in range(KT):
                    msl = slice(mt * 128, (mt + 1) * 128)
                    ps = psB.tile([128, 1536], F32, tag="macc", bufs=2,
                                  name="m2ip")
                    for t in range(12):
                        for (o, w, po) in IMG_CH4:
                            nc.tensor.matmul(ps[:, po:po + w],
                                             wi2_sb[t][:, :, msl],
                                             h_pair[t][:, :, o:o + w],
                                             start=(t == 0), stop=(t == 11),
                                             perf_mode=DR)
                    for b in range(BL):
                        xs = x_sb[mt][:, b * N + TXT:(b + 1) * N]
                        nc.vector.affine_then_add(
                            xs, ps[:, b * 576:(b + 1) * 576], xs,
                            scale=cvec("g2", mt), bias=cvec("g2bi2", mt))
                    # this channel tile of x is final: store it
                    for b in range(BL):
                        nc.sync.dma_start(
                            out=out_d[b, mt * 128:(mt + 1) * 128, :],
                            in_=x_sb[mt][:, b * N:(b + 1) * N])

    nc.compile()
    return nc


# ---------------------------------------------------------- host-side prep
def _prep_inputs(inputs):
    f = lambda k: np.asarray(inputs[k], dtype=np.float32)
    x = f("x")
    rel_bias = f("rel_bias")
    w_qkv = f("w_qkv")
    ln1_g, ln1_b = f("ln1_g"), f("ln1_b")
    q_bias, v_bias = f("q_bias"), f("v_bias")
    w_proj, b_proj = f("w_proj"), f("b_proj")
    gamma1, gamma2 = f("gamma1"), f("gamma2")

    Wq = w_qkv[:C] * ln1_g[None, :]
    Wk = w_qkv[C:2 * C] * ln1_g[None, :]
    Wv = w_qkv[2 * C:] * ln1_g[None, :]
    bq = q_bias + w_qkv[:C] @ ln1_b
    bk = w_qkv[C:2 * C] @ ln1_b
    bv = v_bias + w_qkv[2 * C:] @ ln1_b
    Wq *= SCALE
    bq *= SCALE
    b_projp = b_proj + w_proj @ bv

    wqk = np.concatenate([Wq, Wk], axis=0).T          # [C, 1536]
    wqk_s = wqk.copy()
    wqk_s[:, :C] *= SW_Q
    wqk_s[:, C:] *= SW_K
    wv = Wv.T * SW_V                                  # [C, C]
    wproj = w_proj.T * SW_P                           # [C, C]

    def mlp_fold(w1, b1, w2, b2, g, bb):
        w1f = w1 * g[None, :]
        b1f = b1 + w1 @ bb
        return w1f.T, b1f, w2.T, b2

    wt1, bt1, wt2, bt2 = mlp_fold(f("wt1"), f("bt1"), f("wt2"), f("bt2"),
                                  f("ln2t_g"), f("ln2t_b"))
    wi1, bi1, wi2, bi2 = mlp_fold(f("wi1"), f("bi1"), f("wi2"), f("bi2"),
                                  f("ln2i_g"), f("ln2i_b"))

    gamma1_s = gamma1 / SW_P
    gamma2 = gamma2 / 16.0  # mlp weights are pre-scaled by 16 for fp8
    consts = np.zeros((128, NCONST), np.float32)
    bqk = np.concatenate([bq, bk])
    for i in range(MT_QK):
        consts[:, CCOL["bqk"] + i] = bqk[i * 128:(i + 1) * 128]
    for i in range(KT):
        sl = slice(i * 128, (i + 1) * 128)
        consts[:, CCOL["g1"] + i] = gamma1_s[sl]
        consts[:, CCOL["g1bp"] + i] = (gamma1 * b_projp)[sl]
        consts[:, CCOL["g2"] + i] = gamma2[sl]
        consts[:, CCOL["g2bt2"] + i] = (16.0 * gamma2 * bt2)[sl]
        consts[:, CCOL["g2bi2"] + i] = (16.0 * gamma2 * bi2)[sl]
    for i in range(KT_HID):
        sl = slice(i * 128, (i + 1) * 128)
        consts[:, CCOL["bt1"] + i] = bt1[sl]
        consts[:, CCOL["bi1"] + i] = bi1[sl]

    # raw rel_bias transposed to [H, key, query], keys padded to 640
    ebt = rel_bias.transpose(0, 2, 1)
    eb = np.zeros((H, NKT * 128, N), np.float32)
    eb[:, :N, :] = ebt
    eb = eb.reshape(H, NKT, 128, N)

    bf = lambda a: np.ascontiguousarray(a, dtype=np.float32).astype(bfloat16)
    f8r = lambda a: np.ascontiguousarray(a).astype(ml_dtypes.float8_e4m3)
    f8x = lambda a: np.ascontiguousarray(
        np.asarray(a, dtype=np.float32) * 16.0).astype(ml_dtypes.float8_e4m3)

    def drpair(a, npair, width):           # [C, width] -> [npair,128,2,width]
        return np.ascontiguousarray(
            f8r(a).reshape(npair, 2, 128, width).transpose(0, 2, 1, 3))

    wi1_dr = f8x(wi1).reshape(3, 2, 128, HID).transpose(0, 2, 1, 3)
    wi2_dr = f8x(wi2).reshape(12, 2, 128, C).transpose(0, 2, 1, 3)
    shared = {
        "relbias": bf(eb),
        "id128": np.ascontiguousarray(np.eye(128, dtype=np.float32)
                                      ).astype(bfloat16),
        "wqk": drpair(wqk_s, KTP, 2 * C),
        "wv": drpair(wv, KTP, C),
        "wproj": drpair(wproj, KTP, C),
        "wt1": f8x(wt1), "wi1": np.ascontiguousarray(wi1_dr),
        "wt2": f8x(wt2), "wi2": np.ascontiguousarray(wi2_dr),
        "consts": np.ascontiguousarray(consts),
    }
    # per-core x shards, channel-major, bf16
    xs = x.reshape(NCORES, BL, N, C).transpose(0, 1, 3, 2)
    in_maps = []
    for c in range(NCORES):
        m = dict(shared)
        m["x"] = np.ascontiguousarray(xs[c]).astype(bfloat16)
        in_maps.append(m)
    return in_maps


def kernel(**inputs):
    global LAST_EXEC_TIME_NS, LAST_TRACE_PATH
    _install_ntff_hook()
    from concourse.bass_utils import run_bass_kernel_spmd

    if "nc" not in _NC_CACHE:
        _NC_CACHE["nc"] = _build_nc()
    nc = _NC_CACHE["nc"]

    in_maps = _prep_inputs(inputs)
    trace = os.environ.get(_ENV_TRACE, "") == "1"
    import time as _time
    if LAST_EXEC_TIME_NS is not None and trace:
        _time.sleep(2.0)  # let the previous NRT profile session settle
    res = run_bass_kernel_spmd(nc, in_maps, core_ids=list(range(NCORES)),
                               trace=trace)
    LAST_EXEC_TIME_NS = res.exec_time_ns
    if trace and res.instructions_and_trace is not None:
        LAST_TRACE_PATH = res.instructions_and_trace[1]

    out = np.empty((B, N, C), np.float32)
    for c in range(NCORES):
        oc = np.asarray(res.results[c]["out"]).astype(np.float32)  # [BL,C,N]
        out[c * BL:(c + 1) * BL] = oc.transpose(0, 2, 1)
    return out


# revision 11
# speedup vs baseline: 1.3454x; 1.0148x over previous
"""Trainium2 Bass kernel for nn_Block_55448027791422 (dense transformer block).

Strategy: pure data-parallel over batch B=16 across 8 NeuronCores (2 batches
per core), zero collectives.  Activations live on-chip channel-major (C on
partitions, tokens on free dim).  v3 changes vs v2:
  - residual stream x lives in SBUF as bf16 (halves x/out DMA, removes the
    per-chunk bf16 staging copies the LN stats matmuls needed).
  - attn@V really runs fp8 DoubleRow over key-tile pairs (V is stored as
    [128, 2, H*VHW] pair tiles); only the 104-key tail tile runs plain.
  - softmax denominator reciprocal reads the psum row directly (no staging
    copy); V-tile memsets touch only the denominator ones-columns.
  - q/k psum->sbuf copies moved from Scalar to Vector (tensor_scalar with
    per-partition bias), freeing Scalar for the exp stream.
  - image-MLP matmuls write one 3-bank [128, 1536] psum tile per output
    tile (bank-aligned chunks), so GELU / the residual affine run as ONE
    instruction per tile instead of four (Scalar was the mlp1 pacer).
  - HAM rewarm bursts removed (PE stream is dense enough to stay warm).
"""

import os
import sys
import contextlib
import ctypes
import types

import numpy as np

for _p in ("/opt/trn_rl_repo",):
    if _p not in sys.path:
        sys.path.insert(0, _p)

import ml_dtypes

bfloat16 = ml_dtypes.bfloat16

# ---------------------------------------------------------------- constants
B, N, C, H, HD, HID, TXT = 16, 616, 768, 12, 64, 3072, 40
NCORES = 8
BL = B // NCORES            # 2 batches per core
NT = BL * N                 # 1232 tokens per core
KT = C // 128               # 6 channel tiles
KTP = KT // 2               # 3 channel-pair tiles (fp8 DoubleRow)
MT_QK = (2 * C) // 128      # 12 output tiles for q,k
KT_HID = HID // 128         # 24 hidden tiles
EPS = 1e-5
KEY_TILES = [(0, 128), (128, 128), (256, 128), (384, 128), (512, 104)]
NKT = len(KEY_TILES)
SCALE = HD ** -0.5
SW_Q, SW_K, SW_V, SW_P = 128.0, 16.0, 16.0, 16.0   # fp8 weight pre-scales
EXP_SHIFT = 6.5             # probs = exp(score - EXP_SHIFT); cancels in ratio
VHW = 80                    # per-head column pitch in the V tiles (65 used)

# image-token matmul chunks: (token offset, width, psum offset) -- psum
# offsets are chosen so every chunk stays inside one 512-fp32 psum bank and
# the 1152 image columns are contiguous as [2, 576] for one-shot GELU/affine.
IMG_CH4 = [(TXT, 512, 0), (TXT + 512, 64, 512),
           (N + TXT, 448, 576), (N + TXT + 448, 128, 1024)]


def _chunks(total, step=512, base=0):
    out, o = [], 0
    while o < total:
        s = min(step, total - o)
        out.append((base + o, s))
        o += s
    return out


NT_CH = _chunks(NT)                      # [(0,512),(512,512),(1024,208)]
Q_CH = [(0, 512), (512, 104)]            # per-batch query chunks


# const-vector column indexes in the [128, NCONST] consts tensor
def _cc():
    idx = {}
    c = 0
    for name, n in [("bqk", MT_QK), ("g1", KT), ("g1bp", KT),
                    ("g2", KT), ("g2bt2", KT), ("g2bi2", KT),
                    ("bt1", KT_HID), ("bi1", KT_HID)]:
        idx[name] = c
        c += n
    return idx, c


CCOL, NCONST = _cc()

_ENV_TRACE = "BASS_KERNEL_TRACE"
LAST_EXEC_TIME_NS = None
LAST_TRACE_PATH = None


# ------------------------------------------------------- axon profile hook
def _install_ntff_hook():
    """run_bass_kernel_spmd(trace=True) under axon needs antenv.axon_hooks."""
    if "antenv.axon_hooks" in sys.modules:
        return
    so_path = "/opt/axon/libaxon_pjrt.so"
    state = {"h": None}

    def _build():
        try:
            lib = ctypes.CDLL(so_path)
        except OSError:
            return None
        if not hasattr(lib, "axon_start_nrt_profile"):
            return None
        lib.axon_start_nrt_profile.argtypes = [ctypes.POINTER(ctypes.c_int64),
                                               ctypes.c_size_t]
        lib.axon_start_nrt_profile.restype = ctypes.c_int64
        lib.axon_stop_nrt_profile.argtypes = [ctypes.c_char_p]
        lib.axon_stop_nrt_profile.restype = ctypes.c_int64

        @contextlib.contextmanager
        def _hook(output_dir, device_ids):
            import jax
            jax.devices()
            if device_ids:
                ids = (ctypes.c_int64 * len(device_ids))(*device_ids)
                rc = lib.axon_start_nrt_profile(ids, len(device_ids))
            else:
                rc = lib.axon_start_nrt_profile(None, 0)
            if rc != 0:
                raise RuntimeError(f"axon_start_nrt_profile rc={rc}")
            try:
                yield
            finally:
                n = lib.axon_stop_nrt_profile(str(output_dir).encode())
                if n < 0:
                    raise RuntimeError(f"axon_stop_nrt_profile rc={n}")

        return _hook

    def get_axon_ntff_profile_hook():
        if state["h"] is None:
            state["h"] = _build()
        return state["h"]

    mod = types.ModuleType("antenv.axon_hooks")
    mod.get_axon_ntff_profile_hook = get_axon_ntff_profile_hook
    mod.set_axon_ntff_profile_hook = lambda h: state.update(h=h)
    sys.modules["antenv.axon_hooks"] = mod


# ------------------------------------------------------------ graph builder
_NC_CACHE = {}


def _build_nc():
    import concourse.bass as bass  # noqa: F401
    import concourse.mybir as mybir
    import concourse.tile as tile
    from concourse import bacc

    F32 = mybir.dt.float32
    BF16 = mybir.dt.bfloat16
    F8 = mybir.dt.float8e4
    AF = mybir.ActivationFunctionType
    ALU = mybir.AluOpType
    DR = mybir.MatmulPerfMode.DoubleRow

    nc = bacc.Bacc(None, target_bir_lowering=False)
    d = nc.declare_dram_parameter
    x_d = d("x", [BL, C, N], BF16, isOutput=False)
    eb_d = d("relbias", [H, NKT, 128, N], BF16, isOutput=False)
    wqk_d = d("wqk", [KTP, 128, 2, 2 * C], F8, isOutput=False)
    wv_d = d("wv", [KTP, 128, 2, C], F8, isOutput=False)
    wproj_d = d("wproj", [KTP, 128, 2, C], F8, isOutput=False)
    id_d = d("id128", [128, 128], BF16, isOutput=False)
    wt1_d = d("wt1", [C, HID], F8, isOutput=False)
    wi1_d = d("wi1", [3, 128, 2, HID], F8, isOutput=False)
    wt2_d = d("wt2", [HID, C], F8, isOutput=False)
    wi2_d = d("wi2", [12, 128, 2, C], F8, isOutput=False)
    consts_d = d("consts", [128, NCONST], F32, isOutput=False)
    out_d = d("out", [BL, C, N], BF16, isOutput=True)

    def p3(t):
        """[128, NT] view -> [128, BL, N] batch-split view."""
        return t.rearrange("p (b n) -> p b n", b=BL)

    with tile.TileContext(nc) as tc:
        with contextlib.ExitStack() as octx:
            per = octx.enter_context(tc.tile_pool(name="perm", bufs=1))
            # persistent tiles
            consts = per.tile([128, NCONST], F32, tag="consts")

            def cvec(name, i):
                return consts[:, CCOL[name] + i:CCOL[name] + i + 1]

            nc.sync.dma_start(out=consts[:], in_=consts_d[:])
            ones128 = per.tile([128, 128], BF16, tag="ones128")
            nc.vector.memset(ones128[:], 1.0)
            id128 = per.tile([128, 128], BF16, tag="id128")
            nc.sync.dma_start(out=id128[:], in_=id_d[:])
            eps_ap = per.tile([128, 1], F32, tag="epsap")
            nc.vector.memset(eps_ap[:], EPS)
            shift_ap = per.tile([128, 1], F32, tag="shiftap")
            nc.vector.memset(shift_ap[:], -float(EXP_SHIFT))

            x_sb = [per.tile([128, NT], BF16, tag="x", bufs=KT, name=f"x{i}")
                    for i in range(KT)]
            for kt in range(KT):
                for b in range(BL):
                    nc.sync.dma_start(
                        out=x_sb[kt][:, b * N:(b + 1) * N],
                        in_=x_d[b, kt * 128:(kt + 1) * 128, :])

            # fp8 channel-pair activation tiles (DoubleRow operands):
            # pair t holds channel tiles (2t, 2t+1) in slots (0, 1)
            n1p = [per.tile([128, 2, NT], F8, tag="n1p", bufs=KTP,
                            name=f"n1p{i}") for i in range(KTP)]
            op = [per.tile([128, 2, NT], F8, tag="op", bufs=KTP,
                           name=f"op{i}") for i in range(KTP)]
            # V tiles: pair tiles over key tiles (0,1) and (2,3) for fp8
            # DoubleRow attn@V, plus the plain 104-key tail tile.  Per-head
            # pitch VHW; col h*VHW+64 holds the softmax-denominator ones.
            vpP = [[per.tile([128, 2, H * VHW], F8, tag="vpp", bufs=2 * BL,
                             name=f"vpp{b}_{tp}") for tp in range(2)]
                   for b in range(BL)]
            vp4 = [per.tile([128, H * VHW], F8, tag="vp4", bufs=BL,
                            name=f"vp4{b}") for b in range(BL)]

            n2_pair = [per.tile([128, 2, NT], F8, tag="n2p", bufs=KTP,
                                name=f"n2p{i}") for i in range(KTP)]
            n2_sb = [n2_pair[i // 2][:, i % 2, :] for i in range(KT)]

            # LN scratch lives in the outer pool (used by both phases)
            def ln_scratch(tag, w, dt, bufs, name):
                return per.tile([128, 512], dt, tag=tag, bufs=bufs,
                                name=name)[:, :w]

            # ------------------------------------------------ LayerNorm
            def ln_stats(x_tiles, start, w, pair, psm_fn, pse_fn):
                """Stats for one token chunk -> (mn, r) [128, fw] tiles.
                pair=True: b-symmetric range [start,start+w) of BOTH batches
                (within-batch offset), free size 2w.  Else NT-offset."""
                fw = 2 * w if pair else w

                def src(kt):
                    if pair:
                        return p3(x_tiles[kt])[:, :, start:start + w]
                    return x_tiles[kt][:, start:start + w]

                ps_m = psm_fn(fw)
                ps_e = pse_fn(fw)
                for kt in range(KT):
                    sqt = ln_scratch("sqt", fw, BF16, 3, "sqt")
                    sqt_v = (sqt.rearrange("p (b n) -> p b n", b=2)
                             if pair else sqt)
                    if kt % 2 == 0:
                        nc.scalar.activation(sqt_v, src(kt), AF.Square)
                    else:
                        nc.gpsimd.tensor_mul(sqt_v, src(kt), src(kt))
                    nc.tensor.matmul(ps_m, ones128[:], src(kt),
                                     start=(kt == 0), stop=(kt == KT - 1))
                    nc.tensor.matmul(ps_e, ones128[:], sqt,
                                     start=(kt == 0), stop=(kt == KT - 1))
                m2 = ln_scratch("lntmp", fw, F32, 3, "m2")
                nc.scalar.activation(m2, ps_m, AF.Square,
                                     scale=float(C ** -0.5))
                dd = ln_scratch("lntmp", fw, F32, 3, "dd")
                nc.vector.tensor_sub(dd, ps_e, m2)
                s = ln_scratch("lntmp", fw, F32, 3, "s")
                nc.scalar.activation(s, dd, AF.Sqrt, bias=eps_ap[:, 0:1],
                                     scale=float(1.0 / C))
                r = ln_scratch("lnr", fw, F32, 8, "r")
                nc.vector.reciprocal_approx_fast(out=r, in_=s)
                mn = ln_scratch("lnr", fw, F32, 8, "mn")
                nc.scalar.mul(mn, ps_m, float(1.0 / C))
                return mn, r

            def ln_elem(x_tiles, out_writer, start, w, pair, mn, r):
                fw = 2 * w if pair else w

                def src(kt):
                    if pair:
                        return p3(x_tiles[kt])[:, :, start:start + w]
                    return x_tiles[kt][:, start:start + w]

                mn_v = (mn.rearrange("p (b n) -> p b n", b=2)
                        if pair else mn)
                r_v = r.rearrange("p (b n) -> p b n", b=2) if pair else r
                for kt in range(KT):
                    t = ln_scratch("lnt", fw, BF16, 4, "t")
                    tv = (t.rearrange("p (b n) -> p b n", b=2)
                          if pair else t)
                    e1, e2 = ((nc.gpsimd, nc.vector) if kt % 2 == 0
                              else (nc.vector, nc.gpsimd))
                    e1.tensor_sub(tv, src(kt), mn_v)
                    dst = out_writer(kt, start, w, pair)
                    e2.tensor_mul(dst, tv, r_v)

            # ---------------- early pool (qkv / attention / proj / stats)
            with contextlib.ExitStack() as ectx:
                ep = ectx.enter_context(tc.tile_pool(name="early", bufs=1))
                psA = ectx.enter_context(
                    tc.tile_pool(name="psA", bufs=1, space="PSUM"))

                def psS(w=616):
                    return psA.tile([128, 616], F32, tag="S", bufs=2,
                                    name="psS")[:, :w]

                def psACC(w=616):
                    return psA.tile([128, 616], F32, tag="acc", bufs=2,
                                    name="psAcc")[:, :w]

                # early-dying tags first so the mlp pool can reuse space
                wqk_sb = [ep.tile([128, 2, 2 * C], F8, tag="wqk", bufs=KTP,
                                  name=f"wqk{i}") for i in range(KTP)]
                _d2 = ep.tile([128, 2, 624], F8, tag="prp", bufs=8, name="d2")
                _d3 = ep.tile([128, 624], F8, tag="prs", bufs=4, name="d3")
                _d4 = ep.tile([128, N], BF16, tag="eb", bufs=15, name="d4")
                _d5 = ep.tile([1, N], F32, tag="den", bufs=4, name="d5")
                _d6 = ep.tile([64, N], F32, tag="recb", bufs=2, name="d6")
                wv_sb = [ep.tile([128, 2, C], F8, tag="wv", bufs=KTP,
                                 name=f"wv{i}") for i in range(KTP)]
                wproj_sb = [ep.tile([128, 2, C], F8, tag="wproj", bufs=KTP,
                                    name=f"wpj{i}") for i in range(KTP)]
                for t in range(KTP):
                    nc.sync.dma_start(out=wqk_sb[t][:], in_=wqk_d[t])
                    nc.sync.dma_start(out=wv_sb[t][:], in_=wv_d[t])
                for t in range(KTP):
                    nc.sync.dma_start(out=wproj_sb[t][:], in_=wproj_d[t])

                qk_sb = [ep.tile([128, NT], BF16, tag="qk", bufs=MT_QK,
                                 name=f"qk{i}") for i in range(MT_QK)]

                # -------------------------------------------------- LN1
                def n1_writer(kt, start, w, pair):
                    dst = n1p[kt // 2][:, kt % 2, :]
                    if pair:
                        return p3(dst)[:, :, start:start + w]
                    return dst[:, start:start + w]

                for (o, w) in NT_CH:
                    mn, r = ln_stats(x_sb, o, w, False, psS, psACC)
                    ln_elem(x_sb, n1_writer, o, w, False, mn, r)

                # ------------------------------------------------- v matmul
                # ones columns for the softmax denominator (only cols used
                # by the attn@V stationary reads need initialising)
                for b in range(BL):
                    for tp in range(2):
                        for s in range(2):
                            v3s = (vpP[b][tp][:, s, :]
                                   .rearrange("p (h e) -> p h e", e=VHW))
                            nc.vector.memset(v3s[:, :, 64:65], 1.0)
                    v3 = vp4[b].rearrange("p (h e) -> p h e", e=VHW)
                    nc.vector.memset(v3[:104, :, 64:65], 1.0)
                for b in range(BL):
                    for ktl, (koff, ksz) in enumerate(KEY_TILES):
                        toff = b * N + koff
                        if ktl < 4:
                            vt3 = (vpP[b][ktl // 2][:, ktl % 2, :]
                                   .rearrange("p (h e) -> p h e", e=VHW))
                        else:
                            vt3 = vp4[b].rearrange("p (h e) -> p h e", e=VHW)
                        vch = [(0, 512), (512, 256)]
                        pss = [(psS(512) if ci == 0 else psACC(256))[:ksz, :]
                               for ci in range(2)]
                        # t-outer: one stationary load serves both chunks
                        for t in range(KTP):
                            for ci, (o, w) in enumerate(vch):
                                nc.tensor.matmul(
                                    pss[ci], n1p[t][:, :, toff:toff + ksz],
                                    wv_sb[t][:, :, o:o + w],
                                    start=(t == 0), stop=(t == KTP - 1),
                                    perf_mode=DR)
                        for ci, (o, w) in enumerate(vch):
                            nheads = w // 64
                            h0 = o // 64
                            nc.scalar.activation(
                                vt3[:ksz, h0:h0 + nheads, 0:64],
                                pss[ci].rearrange("p (h e) -> p h e", e=64),
                                AF.Copy, scale=float(1.0 / SW_V))

                # ------------------------------------------------ q,k matmul
                def emit_qk(mt, mi):
                    msl = slice(mt * 128, (mt + 1) * 128)
                    sw = SW_Q if mt < KT else SW_K
                    pss = []
                    for ci, (o, w) in enumerate(NT_CH):
                        ps = (psS(w) if (mi + ci) % 2 == 0 else psACC(w))
                        pss.append((ps, o, w))
                    # t-outer: one stationary load serves all three chunks
                    for t in range(KTP):
                        for ps, o, w in pss:
                            nc.tensor.matmul(ps, wqk_sb[t][:, :, msl],
                                             n1p[t][:, :, o:o + w],
                                             start=(t == 0),
                                             stop=(t == KTP - 1),
                                             perf_mode=DR)
                    for ps, o, w in pss:
                        nc.vector.tensor_scalar(
                            out=qk_sb[mt][:, o:o + w], in0=ps,
                            scalar1=float(1.0 / sw),
                            scalar2=cvec("bqk", mt),
                            op0=ALU.mult, op1=ALU.add)

                # only the first head-pair's q/k upfront; the rest are
                # emitted inside the attention loop one pair ahead, so the
                # qkv matmuls fill attention's exp-bound PE stalls
                emit_qk(0, 0)
                emit_qk(6, 1)

                # ------------------------------------------------ attention
                # b-major pair order means a cached eb tile would need to
                # survive 6 units (buffer rotation would recycle it), so
                # rel_bias tiles are loaded fresh per (pair, batch).
                def eb_tile(h, ktl):
                    t = ep.tile([128, N], BF16, tag="eb", bufs=15,
                                name=f"eb{h}_{ktl}")
                    nc.sync.dma_start(out=t[:], in_=eb_d[h, ktl])
                    return t

                def scores_probs_pair(hp, b):
                    """Scores for heads (2hp, 2hp+1) of batch b.  The two
                    heads' q/k live in rows 0-63 / 64-127 of the same tiles,
                    so their qk matmuls land in disjoint PE row groups and
                    run concurrently when issued back-to-back.  psum =
                    rel_bias (identity matmul) + q.k; praw = exp(s-shift)
                    as fp8 pairs, one set per head."""
                    qt = qk_sb[hp]
                    kt_t = qk_sb[KT + hp]
                    prz = []
                    for _ in range(2):
                        prp = [ep.tile([128, 2, 624], F8, tag="prp", bufs=8,
                                       name="prp") for _ in range(2)]
                        prs = ep.tile([128, 624], F8, tag="prs", bufs=4,
                                      name="prs")
                        prz.append((prp, prs))
                    for ktl, (koff, ksz) in enumerate(KEY_TILES):
                        pz = [psS(), psS()]
                        ebz = [eb_tile(2 * hp, ktl),
                               eb_tile(2 * hp + 1, ktl)]
                        ksl = slice(b * N + koff, b * N + koff + ksz)
                        for i in range(2):
                            for (qo, qw) in Q_CH:
                                nc.tensor.matmul(
                                    pz[i][:ksz, qo:qo + qw], id128[:, :ksz],
                                    ebz[i][:, qo:qo + qw],
                                    start=True, stop=False)
                        # adjacent qk matmuls: rows 0-63 then 64-127
                        for i in range(2):
                            po = i * 64
                            for (qo, qw) in Q_CH:
                                nc.tensor.matmul(
                                    pz[i][:ksz, qo:qo + qw],
                                    kt_t[po:po + 64, ksl],
                                    qt[po:po + 64,
                                       b * N + qo:b * N + qo + qw],
                                    start=False, stop=True)
                        for i in range(2):
                            prp, prs = prz[i]
                            dst = (prp[ktl // 2][:, ktl % 2, :N] if ktl < 4
                                   else prs[:, :N])
                            nc.scalar.activation(dst[:ksz, :],
                                                 pz[i][:ksz, :], AF.Exp,
                                                 bias=shift_ap[:ksz, 0:1])
                    return prz

                def attnv(h, b, prp, prs):
                    vcol = slice(h * VHW, h * VHW + 65)
                    pv = psACC()[:65, :]
                    # fp8 DoubleRow over key-tile pairs; 104-key tail plain
                    for tp in range(2):
                        for (qo, qw) in Q_CH:
                            nc.tensor.matmul(pv[:, qo:qo + qw],
                                             vpP[b][tp][:, :, vcol],
                                             prp[tp][:, :, qo:qo + qw],
                                             start=(tp == 0), stop=False,
                                             perf_mode=DR)
                    for (qo, qw) in Q_CH:
                        nc.tensor.matmul(pv[:, qo:qo + qw],
                                         vp4[b][:104, vcol],
                                         prs[:104, qo:qo + qw],
                                         start=False, stop=True)
                    return pv

                def div_out(h, b, pv):
                    po = (h % 2) * 64
                    kt = h // 2
                    den = ep.tile([1, N], F32, tag="den", bufs=4, name="den")
                    nc.vector.tensor_copy(den[0:1, :], pv[64:65, :])
                    rec = ep.tile([1, N], F32, tag="den", bufs=4, name="rec")
                    nc.vector.reciprocal_approx_fast(out=rec, in_=den)
                    recb = ep.tile([64, N], F32, tag="recb", bufs=2,
                                   name="recb")
                    nc.gpsimd.partition_broadcast(recb[:], rec[0:1, :])
                    nc.vector.tensor_mul(
                        op[kt // 2][po:po + 64, kt % 2,
                                    b * N:(b + 1) * N],
                        pv[0:64, :], recb[:])

                def avdiv(hp, b, prz):
                    for i in range(2):
                        prp, prs = prz[i]
                        pv = attnv(2 * hp + i, b, prp, prs)
                        div_out(2 * hp + i, b, pv)

                def emit_proj(b):
                    for mt in range(KT):
                        msl = slice(mt * 128, (mt + 1) * 128)
                        pss = []
                        for ci, (qo, qw) in enumerate(Q_CH):
                            ps = (psS(qw) if (mt + ci) % 2 == 0
                                  else psACC(qw))
                            pss.append((ps, qo, qw))
                        for t in range(KTP):
                            for ps, qo, qw in pss:
                                nc.tensor.matmul(
                                    ps, wproj_sb[t][:, :, msl],
                                    op[t][:, :, b * N + qo:b * N + qo + qw],
                                    start=(t == 0), stop=(t == KTP - 1),
                                    perf_mode=DR)
                        for ps, qo, qw in pss:
                            xs = x_sb[mt][:, b * N + qo:b * N + qo + qw]
                            nc.vector.affine_then_add(
                                xs, ps, xs, scale=cvec("g1", mt),
                                bias=cvec("g1bp", mt))

                # b-major: all b=0 pairs first, so proj(b=0) overlaps the
                # b=1 attention stream (qkv emission fills the b=0 stream)
                pairs = [(hp, b) for b in range(BL) for hp in range(H // 2)]
                prevu = None
                for ui, u in enumerate(pairs):
                    pr = scores_probs_pair(*u)
                    if prevu is not None:
                        avdiv(prevu[0][0], prevu[0][1], prevu[1])
                        if prevu[0] == (H // 2 - 1, 0):
                            emit_proj(0)
                    prevu = (u, pr)
                    hp, b = u
                    if b == 0 and hp + 1 < H // 2:
                        emit_qk(hp + 1, 0)
                        emit_qk(hp + 7, 1)
                avdiv(prevu[0][0], prevu[0][1], prevu[1])
                emit_proj(1)

                # ------------------------------------- LN2 stats (+text n2)
                def n2_writer(kt, start, w, pair):
                    dst = n2_sb[kt]
                    if pair:
                        return p3(dst)[:, :, start:start + w]
                    return dst[:, start:start + w]

                # text tokens first so the text MLP can start early
                mn_t, r_t = ln_stats(x_sb, 0, TXT, True, psS, psACC)
                ln_elem(x_sb, n2_writer, 0, TXT, True, mn_t, r_t)
                # image stats now (psA still open); elementwise runs in the
                # MLP phase on vector/gpsimd, overlapped with the text MLP
                LN2I = [(TXT, 512, False), (N + TXT, 512, False),
                        (TXT + 512, 64, True)]
                ln2i_mr = [ln_stats(x_sb, o, w, pair, psS, psACC)
                           for (o, w, pair) in LN2I]

            # -------------------------------------------------- MLP phase
            with contextlib.ExitStack() as mctx:
                mp = mctx.enter_context(tc.tile_pool(name="mlp", bufs=1))
                psB = mctx.enter_context(
                    tc.tile_pool(name="psB", bufs=1, space="PSUM"))

                h_pair = [mp.tile([128, 2, NT], F8, tag="h", bufs=KT_HID // 2,
                                  name=f"h{i}") for i in range(KT_HID // 2)]

                wt1_sb = [mp.tile([128, HID], F8, tag="w1", bufs=KT,
                                  name=f"wt1_{i}") for i in range(KT)]
                for kt in range(KT):
                    nc.sync.dma_start(out=wt1_sb[kt][:],
                                      in_=wt1_d[kt * 128:(kt + 1) * 128, :])
                wt2_sb = [mp.tile([128, C], F8, tag="w2", bufs=KT_HID,
                                  name=f"wt2_{i}") for i in range(KT_HID)]
                for kt in range(KT_HID):
                    nc.sync.dma_start(out=wt2_sb[kt][:],
                                      in_=wt2_d[kt * 128:(kt + 1) * 128, :])

                wi1_sb = [mp.tile([128, 2, HID], F8, tag="w1", bufs=KT,
                                  name=f"wi1_{i}") for i in range(3)]
                for t in range(3):
                    nc.sync.dma_start(out=wi1_sb[t][:], in_=wi1_d[t])
                wi2_sb = [mp.tile([128, 2, C], F8, tag="w2", bufs=KT_HID,
                                  name=f"wi2_{i}") for i in range(12)]
                for t in range(12):
                    nc.sync.dma_start(out=wi2_sb[t][:], in_=wi2_d[t])

                def n2_writer2(kt, start, w, pair):
                    dst = n2_sb[kt]
                    if pair:
                        return p3(dst)[:, :, start:start + w]
                    return dst[:, start:start + w]

                # image-token LN2 elementwise (vector/gpsimd; overlaps the
                # text MLP running on PE/scalar)
                for (o, w, pair), (mn_, r_) in zip(LN2I, ln2i_mr):
                    ln_elem(x_sb, n2_writer2, o, w, pair, mn_, r_)

                def n2txt(kt):
                    return p3(n2_sb[kt])[:, :, 0:TXT]

                def htxt(kt):
                    return p3(h_pair[kt // 2][:, kt % 2, :])[:, :, 0:TXT]

                # ---- text mlp1 (plain fp8 matmuls, N=80 strided); runs on
                # the PE while the LN2-image elementwise tail finishes
                for mt in range(KT_HID):
                    msl = slice(mt * 128, (mt + 1) * 128)
                    ps = psB.tile([128, 512], F32, tag="tacc", bufs=2,
                                  name="m1ps")[:, :TXT * BL]
                    for kt in range(KT):
                        nc.tensor.matmul(ps, wt1_sb[kt][:, msl], n2txt(kt),
                                         start=(kt == 0), stop=(kt == KT - 1))
                    nc.scalar.activation(
                        htxt(mt), ps.rearrange("p (b n) -> p b n", b=BL),
                        AF.Gelu, bias=cvec("bt1", mt), scale=1.0 / 16)

                # ---- image mlp1 (fp8 DoubleRow, K=256 per matmul; one
                # 3-bank psum tile per output tile -> single GELU)
                for mt in range(KT_HID):
                    msl = slice(mt * 128, (mt + 1) * 128)
                    ps = psB.tile([128, 1536], F32, tag="macc", bufs=2,
                                  name="m1ip")
                    for t in range(3):
                        for (o, w, po) in IMG_CH4:
                            nc.tensor.matmul(ps[:, po:po + w],
                                             wi1_sb[t][:, :, msl],
                                             n2_pair[t][:, :, o:o + w],
                                             start=(t == 0), stop=(t == 2),
                                             perf_mode=DR)
                    ps3 = ps[:, :1152].rearrange("p (b n) -> p b n", b=2)
                    hs = (h_pair[mt // 2][:, mt % 2, :]
                          .rearrange("p (b n) -> p b n", b=2)[:, :, TXT:])
                    nc.scalar.activation(hs, ps3, AF.Gelu,
                                         bias=cvec("bi1", mt), scale=1.0 / 16)

                # ---- text mlp2 (PE filler for the mlp1 -> mlp2 barrier)
                for mt in range(KT):
                    msl = slice(mt * 128, (mt + 1) * 128)
                    ps = psB.tile([128, 512], F32, tag="tacc", bufs=2,
                                  name="m2ps")[:, :TXT * BL]
                    for kt in range(KT_HID):
                        nc.tensor.matmul(ps, wt2_sb[kt][:, msl], htxt(kt),
                                         start=(kt == 0),
                                         stop=(kt == KT_HID - 1))
                    for b in range(BL):
                        xs = x_sb[mt][:, b * N:b * N + TXT]
                        nc.vector.affine_then_add(
                            xs, ps[:, b * TXT:(b + 1) * TXT], xs,
                            scale=cvec("g2", mt), bias=cvec("g2bt2", mt))

                # ---- image mlp2
                for mt in range(KT):
                    msl = slice(mt * 128, (mt + 1) * 128)
                    ps = psB.tile([128, 1536], F32, tag="macc", bufs=2,
                                  name="m2ip")
                    for t in range(12):
                        for (o, w, po) in IMG_CH4:
                            nc.tensor.matmul(ps[:, po:po + w],
                                             wi2_sb[t][:, :, msl],
                                             h_pair[t][:, :, o:o + w],
                                             start=(t == 0), stop=(t == 11),
                                             perf_mode=DR)
                    for b in range(BL):
                        xs = x_sb[mt][:, b * N + TXT:(b + 1) * N]
                        nc.vector.affine_then_add(
                            xs, ps[:, b * 576:(b + 1) * 576], xs,
                            scale=cvec("g2", mt), bias=cvec("g2bi2", mt))
                    # this channel tile of x is final: store it
                    for b in range(BL):
                        nc.sync.dma_start(
                            out=out_d[b, mt * 128:(mt + 1) * 128, :],
                            in_=x_sb[mt][:, b * N:(b + 1) * N])

    nc.compile()
    return nc


# ---------------------------------------------------------- host-side prep
def _prep_inputs(inputs):
    f = lambda k: np.asarray(inputs[k], dtype=np.float32)
    x = f("x")
    rel_bias = f("rel_bias")
    w_qkv = f("w_qkv")
    ln1_g, ln1_b = f("ln1_g"), f("ln1_b")
    q_bias, v_bias = f("q_bias"), f("v_bias")
    w_proj, b_proj = f("w_proj"), f("b_proj")
    gamma1, gamma2 = f("gamma1"), f("gamma2")

    Wq = w_qkv[:C] * ln1_g[None, :]
    Wk = w_qkv[C:2 * C] * ln1_g[None, :]
    Wv = w_qkv[2 * C:] * ln1_g[None, :]
    bq = q_bias + w_qkv[:C] @ ln1_b
    bk = w_qkv[C:2 * C] @ ln1_b
    bv = v_bias + w_qkv[2 * C:] @ ln1_b
    Wq *= SCALE
    bq *= SCALE
    b_projp = b_proj + w_proj @ bv

    wqk = np.concatenate([Wq, Wk], axis=0).T          # [C, 1536]
    wqk_s = wqk.copy()
    wqk_s[:, :C] *= SW_Q
    wqk_s[:, C:] *= SW_K
    wv = Wv.T * SW_V                                  # [C, C]
    wproj = w_proj.T * SW_P                           # [C, C]

    def mlp_fold(w1, b1, w2, b2, g, bb):
        w1f = w1 * g[None, :]
        b1f = b1 + w1 @ bb
        return w1f.T, b1f, w2.T, b2

    wt1, bt1, wt2, bt2 = mlp_fold(f("wt1"), f("bt1"), f("wt2"), f("bt2"),
                                  f("ln2t_g"), f("ln2t_b"))
    wi1, bi1, wi2, bi2 = mlp_fold(f("wi1"), f("bi1"), f("wi2"), f("bi2"),
                                  f("ln2i_g"), f("ln2i_b"))

    gamma1_s = gamma1 / SW_P
    gamma2 = gamma2 / 16.0  # mlp weights are pre-scaled by 16 for fp8
    consts = np.zeros((128, NCONST), np.float32)
    bqk = np.concatenate([bq, bk])
    for i in range(MT_QK):
        consts[:, CCOL["bqk"] + i] = bqk[i * 128:(i + 1) * 128]
    for i in range(KT):
        sl = slice(i * 128, (i + 1) * 128)
        consts[:, CCOL["g1"] + i] = gamma1_s[sl]
        consts[:, CCOL["g1bp"] + i] = (gamma1 * b_projp)[sl]
        consts[:, CCOL["g2"] + i] = gamma2[sl]
        consts[:, CCOL["g2bt2"] + i] = (16.0 * gamma2 * bt2)[sl]
        consts[:, CCOL["g2bi2"] + i] = (16.0 * gamma2 * bi2)[sl]
    for i in range(KT_HID):
        sl = slice(i * 128, (i + 1) * 128)
        consts[:, CCOL["bt1"] + i] = bt1[sl]
        consts[:, CCOL["bi1"] + i] = bi1[sl]

    # raw rel_bias transposed to [H, key, query], keys padded to 640
    ebt = rel_bias.transpose(0, 2, 1)
    eb = np.zeros((H, NKT * 128, N), np.float32)
    eb[:, :N, :] = ebt
    eb = eb.reshape(H, NKT, 128, N)

    bf = lambda a: np.ascontiguousarray(a, dtype=np.float32).astype(bfloat16)
    f8r = lambda a: np.ascontiguousarray(a).astype(ml_dtypes.float8_e4m3)
    f8x = lambda a: np.ascontiguousarray(
        np.asarray(a, dtype=np.float32) * 16.0).astype(ml_dtypes.float8_e4m3)

    def drpair(a, npair, width):           # [C, width] -> [npair,128,2,width]
        return np.ascontiguousarray(
            f8r(a).reshape(npair, 2, 128, width).transpose(0, 2, 1, 3))

    wi1_dr = f8x(wi1).reshape(3, 2, 128, HID).transpose(0, 2, 1, 3)
    wi2_dr = f8x(wi2).reshape(12, 2, 128, C).transpose(0, 2, 1, 3)
    shared = {
        "relbias": bf(eb),
        "id128": np.ascontiguousarray(np.eye(128, dtype=np.float32)
                                      ).astype(bfloat16),
        "wqk": drpair(wqk_s, KTP, 2 * C),
        "wv": drpair(wv, KTP, C),
        "wproj": drpair(wproj, KTP, C),
        "wt1": f8x(wt1), "wi1": np.ascontiguousarray(wi1_dr),
        "wt2": f8x(wt2), "wi2": np.ascontiguousarray(wi2_dr),
        "consts": np.ascontiguousarray(consts),
    }
    # per-core x shards, channel-major, bf16
    xs = x.reshape(NCORES, BL, N, C).transpose(0, 1, 3, 2)
    in_maps = []
    for c in range(NCORES):
        m = dict(shared)
        m["x"] = np.ascontiguousarray(xs[c]).astype(bfloat16)
        in_maps.append(m)
    return in_maps


def kernel(**inputs):
    global LAST_EXEC_TIME_NS, LAST_TRACE_PATH
    _install_ntff_hook()
    from concourse.bass_utils import run_bass_kernel_spmd

    if "nc" not in _NC_CACHE:
        _NC_CACHE["nc"] = _build_nc()
    nc = _NC_CACHE["nc"]

    in_maps = _prep_inputs(inputs)
    trace = os.environ.get(_ENV_TRACE, "") == "1"
    import time as _time
    if LAST_EXEC_TIME_NS is not None and trace:
        _time.sleep(2.0)  # let the previous NRT profile session settle
    res = run_bass_kernel_spmd(nc, in_maps, core_ids=list(range(NCORES)),
                               trace=trace)
    LAST_EXEC_TIME_NS = res.exec_time_ns
    if trace and res.instructions_and_trace is not None:
        LAST_TRACE_PATH = res.instructions_and_trace[1]

    out = np.empty((B, N, C), np.float32)
    for c in range(NCORES):
        oc = np.asarray(res.results[c]["out"]).astype(np.float32)  # [BL,C,N]
        out[c * BL:(c + 1) * BL] = oc.transpose(0, 2, 1)
    return out
